# revision 1
# baseline (speedup 1.0000x reference)
"""Trainium2 Bass kernel for the NeuralVolatilityModel recurrence.

Strategy
--------
Data-parallel over batch (dim 1 of x): 8 cores x 32 batch each. The time
loop (256 steps) is sequential per core. States are kept dim-major in SBUF
([hidden_dim, batch]) so every RNN GEMM is `out[o, b] = W[o, i] @ h[i, b]`
with lhsT = W^T (stationary weights, prepared host-side in fp16) and
rhs = state (32 columns). Outputs land in PSUM dim-major, so the state
layout is self-consistent across steps: no transposes in the loop.

Host-side prep removes all on-chip data shuffling:
  * x, prev_x (with tmp0 folded in), noise_z are pre-transposed to
    [t, dim, batch]; noise_x is pre-scaled by exp(b_xm).
  * weights are transposed/concatenated; RNN biases are folded into an
    extra all-ones contraction row where a K slot is free, the rest are
    applied via the activation bias operand.

Per step and core, PE runs 26 LDWEIGHTS+MATMUL pairs (fp16, fp32 accum),
ACT runs tanh/exp, DVE does the reparameterization epilogues.
"""

import numpy as np

import concourse.bass as bass
import concourse.tile as tile
from concourse import bacc, mybir
from concourse import bass_utils
from concourse.bass import ts

F16 = mybir.dt.float16
F32 = mybir.dt.float32

N = 256          # time steps == full batch
D = 64           # input dim == latent dim
H = 256          # hidden dim
NCORES = 8
B = N // NCORES  # batch per core = 32

_CACHE = {}


def build_bass(T=N, CT=32, reps=1):
    """Build the Bass module for T time steps, chunked CT steps per loop
    iteration. reps>1 re-runs the whole computation (for device timing by
    slope). Returns the compiled Bacc object."""
    assert T % CT == 0 and CT % 2 == 0
    nchunks = T // CT

    nc = bacc.Bacc("TRN2", target_bir_lowering=False, debug=False,
                   enable_asserts=False, num_devices=NCORES)

    # ---- DRAM I/O (per-core shapes) ----
    d_xT = nc.dram_tensor("xT", [T, D, B], F16, kind="ExternalInput").ap()
    d_xpT = nc.dram_tensor("xpT", [T, D, B], F16, kind="ExternalInput").ap()
    d_nzT = nc.dram_tensor("nzT", [T, D, B], F32, kind="ExternalInput").ap()
    d_nx = nc.dram_tensor("nx", [T, B, D], F32, kind="ExternalInput").ap()

    d_Wxh = nc.dram_tensor("Wxh", [D + 1, H], F16, kind="ExternalInput").ap()
    d_Whh_xh = nc.dram_tensor("Whh_xh", [H, H], F16, kind="ExternalInput").ap()
    d_Whz1 = nc.dram_tensor("Whz1", [H, H], F16, kind="ExternalInput").ap()
    d_Whz2 = nc.dram_tensor("Whz2", [D + 1, H], F16, kind="ExternalInput").ap()
    d_Whh_hz = nc.dram_tensor("Whh_hz", [H, H], F16, kind="ExternalInput").ap()
    d_Wzmzl = nc.dram_tensor("Wzmzl", [H, 2 * D], F16, kind="ExternalInput").ap()
    d_Wzx1 = nc.dram_tensor("Wzx1", [D + 1, H], F16, kind="ExternalInput").ap()
    d_Wzx2 = nc.dram_tensor("Wzx2", [D, H], F16, kind="ExternalInput").ap()
    d_Whh_zh = nc.dram_tensor("Whh_zh", [H, H], F16, kind="ExternalInput").ap()
    d_Wxm = nc.dram_tensor("Wxm", [H, D], F16, kind="ExternalInput").ap()

    d_bz = nc.dram_tensor("bz", [2 * D, 1], F32, kind="ExternalInput").ap()
    d_bxm = nc.dram_tensor("bxm", [B, D], F32, kind="ExternalInput").ap()

    d_hi0 = nc.dram_tensor("hi0", [128, 2 * B], F16, kind="ExternalInput").ap()
    d_hl0 = nc.dram_tensor("hl0", [128, 2 * B], F16, kind="ExternalInput").ap()
    d_ho0 = nc.dram_tensor("ho0", [128, 2 * B], F16, kind="ExternalInput").ap()
    d_z0 = nc.dram_tensor("z0", [D, B], F16, kind="ExternalInput").ap()

    d_out = nc.dram_tensor("out", [T, B, D], F32, kind="ExternalOutput").ap()

    with tile.TileContext(nc) as tc:
        with (
            tc.tile_pool(name="weights", bufs=1) as wp,
            tc.tile_pool(name="states", bufs=1) as sp,
            tc.tile_pool(name="chunks", bufs=2) as cp,
            tc.tile_pool(name="epil", bufs=3) as ep,
            tc.tile_pool(name="ps2", bufs=2, space="PSUM") as pp2,
            tc.tile_pool(name="ps1", bufs=1, space="PSUM") as pp1,
        ):
            # ---- persistent weights ----
            w_xh = wp.tile([D + 1, H], F16, tag="w_xh")
            nc.sync.dma_start(out=w_xh, in_=d_Wxh)
            # [H, H] weights stored as [128, 2, H]: k-tile index on free dim
            w_hh_xh = wp.tile([128, 2, H], F16, tag="w_hh_xh")
            nc.sync.dma_start(
                out=w_hh_xh, in_=d_Whh_xh.rearrange("(k p) m -> p k m", p=128))
            w_hz1 = wp.tile([128, 2, H], F16, tag="w_hz1")
            nc.sync.dma_start(
                out=w_hz1, in_=d_Whz1.rearrange("(k p) m -> p k m", p=128))
            w_hz2 = wp.tile([D + 1, H], F16, tag="w_hz2")
            nc.sync.dma_start(out=w_hz2, in_=d_Whz2)
            w_hh_hz = wp.tile([128, 2, H], F16, tag="w_hh_hz")
            nc.sync.dma_start(
                out=w_hh_hz, in_=d_Whh_hz.rearrange("(k p) m -> p k m", p=128))
            w_zmzl = wp.tile([128, 2, 2 * D], F16, tag="w_zmzl")
            nc.sync.dma_start(
                out=w_zmzl, in_=d_Wzmzl.rearrange("(k p) m -> p k m", p=128))
            w_zx1 = wp.tile([D + 1, H], F16, tag="w_zx1")
            nc.sync.dma_start(out=w_zx1, in_=d_Wzx1)
            w_zx2 = wp.tile([D, H], F16, tag="w_zx2")
            nc.sync.dma_start(out=w_zx2, in_=d_Wzx2)
            w_hh_zh = wp.tile([128, 2, H], F16, tag="w_hh_zh")
            nc.sync.dma_start(
                out=w_hh_zh, in_=d_Whh_zh.rearrange("(k p) m -> p k m", p=128))
            w_xm = wp.tile([128, 2, D], F16, tag="w_xm")
            nc.sync.dma_start(
                out=w_xm, in_=d_Wxm.rearrange("(k p) m -> p k m", p=128))

            b_z = wp.tile([2 * D, 1], F32, tag="b_z")
            nc.sync.dma_start(out=b_z, in_=d_bz)
            b_xm = wp.tile([B, D], F32, tag="b_xm")
            nc.sync.dma_start(out=b_xm, in_=d_bxm)

            # ---- persistent states, parity 0/1 (step t writes t%2) ----
            hi_sb = sp.tile([128, 2, 2 * B], F16, tag="hi_sb")
            hl_sb = sp.tile([128, 2, 2 * B], F16, tag="hl_sb")
            ho_sb = sp.tile([128, 2, 2 * B], F16, tag="ho_sb")
            u_rhs = sp.tile([D + 1, 2, B], F16, tag="u_rhs")     # u | ones
            zo_rhs = sp.tile([D + 1, 2, B], F16, tag="zo_rhs")   # z | ones
            px_rhs = sp.tile([D, 2, B], F16, tag="px_rhs")       # pxT
            xh_rhs = sp.tile([D + 1, 2, B], F16, tag="xh_rhs")   # xT | ones

            nc.vector.memset(u_rhs[D:D + 1, :, :], 1.0)
            nc.vector.memset(zo_rhs[D:D + 1, :, :], 1.0)
            nc.vector.memset(xh_rhs[D:D + 1, :, :], 1.0)

            # initial states -> parity 1 (step 0 reads parity 1)
            nc.sync.dma_start(out=hi_sb[:, 1, :], in_=d_hi0)
            nc.sync.dma_start(out=hl_sb[:, 1, :], in_=d_hl0)
            nc.sync.dma_start(out=ho_sb[:, 1, :], in_=d_ho0)
            nc.sync.dma_start(out=u_rhs[0:D, 1, :], in_=d_z0)

            chunk_tiles = {}

            def load_chunk(c):
                sl = slice(c * CT, (c + 1) * CT)
                cx = cp.tile([D, CT, B], F16, tag="c_xT")
                nc.sync.dma_start(out=cx,
                                  in_=d_xT[sl].rearrange("t d b -> d t b"))
                cxp = cp.tile([D, CT, B], F16, tag="c_xpT")
                nc.sync.dma_start(out=cxp,
                                  in_=d_xpT[sl].rearrange("t d b -> d t b"))
                cnz = cp.tile([D, CT, B], F32, tag="c_nzT")
                nc.sync.dma_start(out=cnz,
                                  in_=d_nzT[sl].rearrange("t d b -> d t b"))
                cnx = cp.tile([B, CT, D], F32, tag="c_nx")
                nc.sync.dma_start(out=cnx,
                                  in_=d_nx[sl].rearrange("t b d -> b t d"))
                cout = cp.tile([B, CT, D], F32, tag="c_out")
                chunk_tiles[c] = (cx, cxp, cnz, cnx, cout)

            def store_chunk(c):
                sl = slice(c * CT, (c + 1) * CT)
                nc.sync.dma_start(out=d_out[sl].rearrange("t b d -> b t d"),
                                  in_=chunk_tiles[c][4])

            def mm(out_ap, lhsT, rhs, start, stop):
                nc.tensor.matmul(out_ap, lhsT, rhs, start=start, stop=stop)

            def emit_hi(s):
                c, t = divmod(s, CT)
                cx, cxp = chunk_tiles[c][0], chunk_tiles[c][1]
                pc, pp = s % 2, 1 - s % 2
                # stage fresh xT(t) (with ones row at D) and pxT(t)
                nc.vector.tensor_copy(xh_rhs[0:D, pc, :], cx[:, t, :])
                nc.vector.tensor_copy(px_rhs[:, pc, :], cxp[:, t, :])
                ps = pp2.tile([128, 2 * B], F32, tag="hi_ps")
                for m in range(2):
                    o = ps[:, m * B:(m + 1) * B]
                    msl = slice(m * 128, (m + 1) * 128)
                    mm(o, w_hh_xh[:, 0, msl], hi_sb[:, pp, 0:B], True, False)
                    mm(o, w_hh_xh[:, 1, msl], hi_sb[:, pp, B:2 * B], False, False)
                    mm(o, w_xh[:, msl], xh_rhs[:, pc, :], False, True)
                nc.scalar.activation(hi_sb[:, pc, :], ps,
                                     mybir.ActivationFunctionType.Tanh)

            def emit_hl(s):
                # Late-dependency matmuls (u(t-1), fresh tanh-hi(t)) go
                # last so the ready ones fill the PE while they resolve.
                pc, pp = s % 2, 1 - s % 2
                ps = pp2.tile([128, 2 * B], F32, tag="hl_ps")
                for m in range(2):
                    o = ps[:, m * B:(m + 1) * B]
                    msl = slice(m * 128, (m + 1) * 128)
                    mm(o, w_hh_hz[:, 0, msl], hl_sb[:, pp, 0:B], True, False)
                    mm(o, w_hh_hz[:, 1, msl], hl_sb[:, pp, B:2 * B], False, False)
                    mm(o, w_hz2[:, msl], u_rhs[:, pp, :], False, False)
                    mm(o, w_hz1[:, 0, msl], hi_sb[:, pc, 0:B], False, False)
                    mm(o, w_hz1[:, 1, msl], hi_sb[:, pc, B:2 * B], False, True)
                nc.scalar.activation(hl_sb[:, pc, :], ps,
                                     mybir.ActivationFunctionType.Tanh)

            def emit_z(s):
                c, t = divmod(s, CT)
                cnz = chunk_tiles[c][2]
                pc = s % 2
                ps = pp1.tile([128, B], F32, tag="zt_ps")
                mm(ps, w_zmzl[:, 0, :], hl_sb[:, pc, 0:B], True, False)
                mm(ps, w_zmzl[:, 1, :], hl_sb[:, pc, B:2 * B], False, True)
                # u = exp(lv + b_zl) * nz feeds the (rewritten) hl
                # recurrence; full z = u + (mz + b_zm) only feeds ho.
                ez = ep.tile([D, B], F32, tag="ez")
                nc.scalar.activation(ez, ps[D:2 * D, :],
                                     mybir.ActivationFunctionType.Exp,
                                     bias=b_z[D:2 * D, :])
                nc.vector.tensor_mul(u_rhs[0:D, pc, :], ez, cnz[:, t, :])
                nc.vector.scalar_tensor_tensor(
                    zo_rhs[0:D, pc, :], ps[0:D, :], b_z[0:D, :],
                    u_rhs[0:D, pc, :],
                    mybir.AluOpType.add, mybir.AluOpType.add)

            def emit_ho(s):
                pc, pp = s % 2, 1 - s % 2
                ps = pp2.tile([128, 2 * B], F32, tag="ho_ps")
                for m in range(2):
                    o = ps[:, m * B:(m + 1) * B]
                    msl = slice(m * 128, (m + 1) * 128)
                    mm(o, w_hh_zh[:, 0, msl], ho_sb[:, pp, 0:B], True, False)
                    mm(o, w_hh_zh[:, 1, msl], ho_sb[:, pp, B:2 * B], False, False)
                    mm(o, w_zx2[:, msl], px_rhs[:, pc, :], False, False)
                    mm(o, w_zx1[:, msl], zo_rhs[:, pc, :], False, True)
                nc.scalar.activation(ho_sb[:, pc, :], ps,
                                     mybir.ActivationFunctionType.Tanh)

            def emit_xpred(s):
                c, t = divmod(s, CT)
                cnx, cout = chunk_tiles[c][3], chunk_tiles[c][4]
                pc = s % 2
                ps = pp1.tile([B, D], F32, tag="mx_ps")
                mm(ps, ho_sb[:, pc, 0:B], w_xm[:, 0, :], True, False)
                mm(ps, ho_sb[:, pc, B:2 * B], w_xm[:, 1, :], False, True)
                # x_pred = exp(mx)*nx' + (mx + b_xm); nx' pre-scaled by
                # exp(b_xm) on the host.
                ex = ep.tile([B, D], F32, tag="ex")
                nc.scalar.activation(ex, ps,
                                     mybir.ActivationFunctionType.Exp)
                # m2 reads PSUM (GPSIMD can't); products go to GPSIMD to
                # keep the DVE queue free for next-step staging copies.
                m2 = ep.tile([B, D], F32, tag="m2")
                nc.vector.tensor_add(m2, ps, b_xm)
                p1 = ep.tile([B, D], F32, tag="p1")
                nc.gpsimd.tensor_mul(p1, ex, cnx[:, t, :])
                nc.gpsimd.tensor_add(cout[:, t, :], p1, m2)

            from contextlib import ExitStack
            with ExitStack() as stk:
                if reps > 1:
                    stk.enter_context(tc.For_i(0, reps, 1))
                chunk_tiles.clear()
                load_chunk(0)
                for s in range(T):
                    c, t = divmod(s, CT)
                    if t == CT // 2 and c + 1 < nchunks:
                        load_chunk(c + 1)
                    emit_hi(s)
                    emit_hl(s)
                    emit_z(s)
                    if s > 0:
                        emit_ho(s - 1)
                        emit_xpred(s - 1)
                        if s % CT == 0:
                            store_chunk(c - 1)
                emit_ho(T - 1)
                emit_xpred(T - 1)
                store_chunk(nchunks - 1)

    nc.compile()
    return nc


def prep_inputs(x, h_in0, h_lat0, h_out0, z0, tmp0, noise_z, noise_x,
                W_xh_ih, b_xh_ih, W_xh_hh, b_xh_hh,
                W_hz_ih, b_hz_ih, W_hz_hh, b_hz_hh,
                W_zh_ih, b_zh_ih, W_zh_hh, b_zh_hh,
                W_zm, b_zm, W_zl, b_zl, W_xm, b_xm, T=N):
    """Host-side preprocessing; returns the per-core in_map list."""
    f16, f32 = np.float16, np.float32
    xprev = np.concatenate([tmp0[None], x[:-1]], axis=0)
    xT = np.ascontiguousarray(x.transpose(0, 2, 1)).astype(f16)      # [t,d,b]
    xpT = np.ascontiguousarray(xprev.transpose(0, 2, 1)).astype(f16)
    nzT = np.ascontiguousarray(noise_z.transpose(0, 2, 1)).astype(f32)
    nxs = np.ascontiguousarray(noise_x * np.exp(b_xm)[None, None, :]).astype(f32)

    # Rewritten hl recurrence: with z = u + mz + b_zm (u = exp(lv)*nz,
    # mz = hl @ W_zm.T), fold the mz feedback into the hl-hl weight so only
    # u sits on the critical path:
    #   hl' = tanh(hi' @ Wz1.T + u @ Wz2.T
    #              + hl @ (W_hz_hh + Wz2 @ W_zm).T + b_hz + Wz2 @ b_zm)
    Wz2 = W_hz_ih[:, H:]
    b_hz_eff = b_hz_ih + b_hz_hh + Wz2 @ b_zm
    shared = {
        "Wxh": np.concatenate([W_xh_ih.T, (b_xh_ih + b_xh_hh)[None, :]],
                              axis=0).astype(f16),
        "Whh_xh": np.ascontiguousarray(W_xh_hh.T).astype(f16),
        "Whz1": np.ascontiguousarray(W_hz_ih[:, :H].T).astype(f16),
        "Whz2": np.concatenate([Wz2.T, b_hz_eff[None, :]],
                               axis=0).astype(f16),
        "Whh_hz": np.ascontiguousarray((W_hz_hh + Wz2 @ W_zm).T).astype(f16),
        "Wzmzl": np.ascontiguousarray(
            np.concatenate([W_zm.T, W_zl.T], axis=1)).astype(f16),
        "Wzx1": np.concatenate(
            [W_zh_ih[:, :D].T, (b_zh_ih + b_zh_hh)[None, :]],
            axis=0).astype(f16),
        "Wzx2": np.ascontiguousarray(W_zh_ih[:, D:].T).astype(f16),
        "Whh_zh": np.ascontiguousarray(W_zh_hh.T).astype(f16),
        "Wxm": np.ascontiguousarray(W_xm.T).astype(f16),
        "bz": np.concatenate([b_zm, b_zl]).astype(f32).reshape(2 * D, 1),
        "bxm": np.broadcast_to(b_xm, (B, D)).astype(f32).copy(),
    }

    def pack_state(h):       # [b_full, H] -> per-core [128, 2*B] packed
        hT = h.T.astype(f16)                     # [H, b_full]
        return hT.reshape(2, 128, h.shape[0])    # [k, p, b]

    hi_p, hl_p, ho_p = pack_state(h_in0), pack_state(h_lat0), pack_state(h_out0)
    # u0 chosen so the rewritten recurrence reproduces the given z0 exactly:
    # u0 = z0 - mz(h_lat0) - b_zm
    u0 = z0 - h_lat0 @ W_zm.T - b_zm
    z0T = u0.T.astype(f16)                       # [D, b_full]

    in_maps = []
    for c in range(NCORES):
        bs = slice(c * B, (c + 1) * B)
        m = dict(shared)
        m["xT"] = np.ascontiguousarray(xT[:T, :, bs])
        m["xpT"] = np.ascontiguousarray(xpT[:T, :, bs])
        m["nzT"] = np.ascontiguousarray(nzT[:T, :, bs])
        m["nx"] = np.ascontiguousarray(nxs[:T, bs, :])
        m["hi0"] = np.ascontiguousarray(
            hi_p[:, :, bs].transpose(1, 0, 2).reshape(128, 2 * B))
        m["hl0"] = np.ascontiguousarray(
            hl_p[:, :, bs].transpose(1, 0, 2).reshape(128, 2 * B))
        m["ho0"] = np.ascontiguousarray(
            ho_p[:, :, bs].transpose(1, 0, 2).reshape(128, 2 * B))
        m["z0"] = np.ascontiguousarray(z0T[:, bs])
        in_maps.append(m)
    return in_maps


def _get_nc(T=N, CT=32):
    key = (T, CT)
    if key not in _CACHE:
        _CACHE[key] = build_bass(T, CT)
    return _CACHE[key]


def run_on_hw(in_maps, T=N, CT=32):
    nc = _get_nc(T, CT)
    res = bass_utils.run_bass_kernel_spmd(
        nc, in_maps, core_ids=list(range(NCORES)))
    return res.results


class Runner:
    """Persistent jitted SPMD executor for a built Bass module (jit traced
    once; subsequent calls only pay H2D + execute)."""

    def __init__(self, nc):
        import jax
        from jax.sharding import Mesh, PartitionSpec, NamedSharding
        from jax.experimental.shard_map import shard_map
        from concourse import bass2jax

        bass2jax.install_neuronx_cc_hook()
        self._jax = jax
        pname = nc.partition_id_tensor.name if nc.partition_id_tensor else None
        in_names, out_names, out_avals, zeros = [], [], [], []
        for alloc in nc.m.functions[0].allocations:
            if not isinstance(alloc, mybir.MemoryLocationSet):
                continue
            name = alloc.memorylocations[0].name
            if alloc.kind == "ExternalInput":
                if name != pname:
                    in_names.append(name)
            elif alloc.kind == "ExternalOutput":
                out_names.append(name)
                shape = tuple(alloc.tensor_shape)
                dtype = mybir.dt.np(alloc.dtype)
                out_avals.append(jax.core.ShapedArray(shape, dtype))
                zeros.append(np.zeros(shape, dtype))
        self.in_names = list(in_names)
        self.out_names = list(out_names)
        all_names = in_names + out_names
        if pname is not None:
            all_names = all_names + [pname]

        def _body(*args):
            operands = list(args)
            if pname is not None:
                operands.append(bass2jax.partition_id_tensor())
            outs = bass2jax._bass_exec_p.bind(
                *operands,
                out_avals=tuple(out_avals),
                in_names=tuple(all_names),
                out_names=tuple(out_names),
                lowering_input_output_aliases=(),
                sim_require_finite=True,
                sim_require_nnan=True,
                nc=nc,
            )
            return tuple(outs)

        self._body = _body
        devices = jax.devices()[:NCORES]
        self.mesh = Mesh(np.asarray(devices), ("core",))
        spec = PartitionSpec("core")
        self.sharding = NamedSharding(self.mesh, spec)
        nin = len(in_names) + len(zeros)
        self.fn = jax.jit(
            shard_map(_body, mesh=self.mesh, in_specs=(spec,) * nin,
                      out_specs=(spec,) * len(out_names), check_rep=False),
            keep_unused=True)
        self.dev_zeros = [
            jax.device_put(np.zeros((NCORES * z.shape[0], *z.shape[1:]),
                                    z.dtype), self.sharding)
            for z in zeros]
        self.out_shapes = [tuple(a.shape) for a in out_avals]

    def concat_inputs(self, in_maps):
        return [np.concatenate([np.asarray(m[n]) for m in in_maps], axis=0)
                for n in self.in_names]

    def stage(self, in_maps):
        return [self._jax.device_put(a, self.sharding)
                for a in self.concat_inputs(in_maps)]

    def __call__(self, staged):
        outs = self.fn(*staged, *self.dev_zeros)
        self._jax.block_until_ready(outs)
        return outs

    def make_loop_fn(self, iters):
        """Jitted fn chaining `iters` kernel executions inside one dispatch
        (for timing: slope over iters = per-exec device time)."""
        import jax
        from jax.experimental.shard_map import shard_map
        from jax.sharding import PartitionSpec

        nx_i = self.in_names.index("nx")
        out_i = self.out_names.index("out")
        nin = len(self.in_names)
        body_fn = self._body

        def _loop(*args):
            ins = list(args[:nin])
            zeros = list(args[nin:])

            def body(i, carry):
                a = list(ins)
                a[nx_i] = a[nx_i] + 0.0 * carry
                outs = body_fn(*a, *zeros)
                return outs[out_i]

            return (jax.lax.fori_loop(0, iters, body, zeros[out_i]),)

        spec = PartitionSpec("core")
        nargs = nin + len(self.dev_zeros)
        return jax.jit(
            shard_map(_loop, mesh=self.mesh, in_specs=(spec,) * nargs,
                      out_specs=(spec,), check_rep=False),
            keep_unused=True)

    def split(self, outs):
        res = []
        for c in range(NCORES):
            res.append({
                n: np.asarray(outs[i]).reshape(NCORES, *self.out_shapes[i])[c]
                for i, n in enumerate(self.out_names)})
        return res


def get_runner(T=N, CT=32):
    key = ("runner", T, CT)
    if key not in _CACHE:
        _CACHE[key] = Runner(_get_nc(T, CT))
    return _CACHE[key]


def kernel(**inputs):
    in_maps = prep_inputs(**inputs)
    r = get_runner()
    outs = r(r.concat_inputs(in_maps))
    results = r.split(outs)
    out = np.empty((N, N, D), dtype=np.float32)
    for c in range(NCORES):
        out[:, c * B:(c + 1) * B, :] = results[c]["out"]
    return out



# revision 3
# speedup vs baseline: 166.3547x; 166.3547x over previous
"""Trainium2 Bass kernel for the NeuralVolatilityModel recurrence.

Strategy
--------
Data-parallel over batch (dim 1 of x): 8 cores x 32 batch each. The time
loop (256 steps) is sequential per core. States are kept dim-major in SBUF
([hidden_dim, batch]) so every RNN GEMM is `out[o, b] = W[o, i] @ h[i, b]`
with lhsT = W^T (stationary weights, prepared host-side in fp16) and
rhs = state (32 columns). Outputs land in PSUM dim-major, so the state
layout is self-consistent across steps: no transposes in the loop.

Host-side prep removes all on-chip data shuffling:
  * x, prev_x (with tmp0 folded in), noise_z are pre-transposed to
    [t, dim, batch]; noise_x is pre-scaled by exp(b_xm).
  * weights are transposed/concatenated; RNN biases are folded into an
    extra all-ones contraction row where a K slot is free, the rest are
    applied via the activation bias operand.

Per step and core, PE runs 26 LDWEIGHTS+MATMUL pairs (fp16, fp32 accum),
ACT runs tanh/exp, DVE does the reparameterization epilogues.

The kernel is latency-bound on the serial z-chain
(tanh_hl -> z GEMM -> exp -> mul -> hl GEMM); the u-dependent matmuls of
the hl GEMM are ordered last so only two matmuls separate u's arrival
from the tanh.
"""

import numpy as np

import concourse.bass as bass
import concourse.tile as tile
from concourse import bacc, mybir
from concourse import bass_utils
from concourse.bass import ts

F16 = mybir.dt.float16
F32 = mybir.dt.float32

N = 256          # time steps == full batch
D = 64           # input dim == latent dim
H = 256          # hidden dim
NCORES = 8
B = N // NCORES  # batch per core = 32

_CACHE = {}


def build_bass(T=N, CT=32, reps=1):
    """Build the Bass module for T time steps, chunked CT steps per loop
    iteration. reps>1 re-runs the whole computation (for device timing by
    slope). Returns the compiled Bacc object."""
    assert T % CT == 0 and CT % 2 == 0
    nchunks = T // CT

    nc = bacc.Bacc("TRN2", target_bir_lowering=False, debug=False,
                   enable_asserts=False, num_devices=NCORES)

    # ---- DRAM I/O (per-core shapes) ----
    d_xT = nc.dram_tensor("xT", [T, D, B], F16, kind="ExternalInput").ap()
    d_xpT = nc.dram_tensor("xpT", [T, D, B], F16, kind="ExternalInput").ap()
    d_nzT = nc.dram_tensor("nzT", [T, D, B], F32, kind="ExternalInput").ap()
    d_nx = nc.dram_tensor("nx", [T, B, D], F32, kind="ExternalInput").ap()

    d_Wxh = nc.dram_tensor("Wxh", [D + 1, H], F16, kind="ExternalInput").ap()
    d_Whh_xh = nc.dram_tensor("Whh_xh", [H, H], F16, kind="ExternalInput").ap()
    d_Whz1 = nc.dram_tensor("Whz1", [H, H], F16, kind="ExternalInput").ap()
    d_Whz2 = nc.dram_tensor("Whz2", [D + 1, H], F16, kind="ExternalInput").ap()
    d_Whh_hz = nc.dram_tensor("Whh_hz", [H, H], F16, kind="ExternalInput").ap()
    d_Wzmzl = nc.dram_tensor("Wzmzl", [H, 2 * D], F16, kind="ExternalInput").ap()
    d_Wzx1 = nc.dram_tensor("Wzx1", [D + 1, H], F16, kind="ExternalInput").ap()
    d_Wzx2 = nc.dram_tensor("Wzx2", [D, H], F16, kind="ExternalInput").ap()
    d_Whh_zh = nc.dram_tensor("Whh_zh", [H, H], F16, kind="ExternalInput").ap()
    d_Wxm = nc.dram_tensor("Wxm", [H, D], F16, kind="ExternalInput").ap()

    d_bz = nc.dram_tensor("bz", [2 * D, 1], F32, kind="ExternalInput").ap()
    d_bxm = nc.dram_tensor("bxm", [B, D], F32, kind="ExternalInput").ap()

    d_hi0 = nc.dram_tensor("hi0", [128, 2 * B], F16, kind="ExternalInput").ap()
    d_hl0 = nc.dram_tensor("hl0", [128, 2 * B], F16, kind="ExternalInput").ap()
    d_ho0 = nc.dram_tensor("ho0", [128, 2 * B], F16, kind="ExternalInput").ap()
    d_z0 = nc.dram_tensor("z0", [D, B], F16, kind="ExternalInput").ap()

    d_out = nc.dram_tensor("out", [T, B, D], F32, kind="ExternalOutput").ap()

    with tile.TileContext(nc) as tc:
        with (
            tc.tile_pool(name="weights", bufs=1) as wp,
            tc.tile_pool(name="states", bufs=1) as sp,
            tc.tile_pool(name="chunks", bufs=2) as cp,
            tc.tile_pool(name="epil", bufs=3) as ep,
            tc.tile_pool(name="ps2", bufs=2, space="PSUM") as pp2,
            tc.tile_pool(name="ps1", bufs=1, space="PSUM") as pp1,
        ):
            # ---- persistent weights ----
            w_xh = wp.tile([D + 1, H], F16, tag="w_xh")
            nc.sync.dma_start(out=w_xh, in_=d_Wxh)
            # [H, H] weights stored as [128, 2, H]: k-tile index on free dim
            w_hh_xh = wp.tile([128, 2, H], F16, tag="w_hh_xh")
            nc.sync.dma_start(
                out=w_hh_xh, in_=d_Whh_xh.rearrange("(k p) m -> p k m", p=128))
            w_hz1 = wp.tile([128, 2, H], F16, tag="w_hz1")
            nc.sync.dma_start(
                out=w_hz1, in_=d_Whz1.rearrange("(k p) m -> p k m", p=128))
            w_hz2 = wp.tile([D + 1, H], F16, tag="w_hz2")
            nc.sync.dma_start(out=w_hz2, in_=d_Whz2)
            w_hh_hz = wp.tile([128, 2, H], F16, tag="w_hh_hz")
            nc.sync.dma_start(
                out=w_hh_hz, in_=d_Whh_hz.rearrange("(k p) m -> p k m", p=128))
            w_zmzl = wp.tile([128, 2, 2 * D], F16, tag="w_zmzl")
            nc.sync.dma_start(
                out=w_zmzl, in_=d_Wzmzl.rearrange("(k p) m -> p k m", p=128))
            w_zx1 = wp.tile([D + 1, H], F16, tag="w_zx1")
            nc.sync.dma_start(out=w_zx1, in_=d_Wzx1)
            w_zx2 = wp.tile([D, H], F16, tag="w_zx2")
            nc.sync.dma_start(out=w_zx2, in_=d_Wzx2)
            w_hh_zh = wp.tile([128, 2, H], F16, tag="w_hh_zh")
            nc.sync.dma_start(
                out=w_hh_zh, in_=d_Whh_zh.rearrange("(k p) m -> p k m", p=128))
            w_xm = wp.tile([128, 2, D], F16, tag="w_xm")
            nc.sync.dma_start(
                out=w_xm, in_=d_Wxm.rearrange("(k p) m -> p k m", p=128))

            b_z = wp.tile([2 * D, 1], F32, tag="b_z")
            nc.sync.dma_start(out=b_z, in_=d_bz)
            b_xm = wp.tile([B, D], F32, tag="b_xm")
            nc.sync.dma_start(out=b_xm, in_=d_bxm)

            # ---- persistent states, parity 0/1 (step t writes t%2) ----
            hi_sb = sp.tile([128, 2, 2 * B], F16, tag="hi_sb")
            hl_sb = sp.tile([128, 2, 2 * B], F16, tag="hl_sb")
            ho_sb = sp.tile([128, 2, 2 * B], F16, tag="ho_sb")
            u_rhs = sp.tile([D + 1, 2, B], F16, tag="u_rhs")     # u | ones
            zo_rhs = sp.tile([D + 1, 2, B], F16, tag="zo_rhs")   # z | ones
            px_rhs = sp.tile([D, 2, B], F16, tag="px_rhs")       # pxT
            xh_rhs = sp.tile([D + 1, 2, B], F16, tag="xh_rhs")   # xT | ones

            nc.vector.memset(u_rhs[D:D + 1, :, :], 1.0)
            nc.vector.memset(zo_rhs[D:D + 1, :, :], 1.0)
            nc.vector.memset(xh_rhs[D:D + 1, :, :], 1.0)

            # initial states -> parity 1 (step 0 reads parity 1)
            nc.sync.dma_start(out=hi_sb[:, 1, :], in_=d_hi0)
            nc.sync.dma_start(out=hl_sb[:, 1, :], in_=d_hl0)
            nc.sync.dma_start(out=ho_sb[:, 1, :], in_=d_ho0)
            nc.sync.dma_start(out=u_rhs[0:D, 1, :], in_=d_z0)

            chunk_tiles = {}

            def load_chunk(c):
                sl = slice(c * CT, (c + 1) * CT)
                cx = cp.tile([D, CT, B], F16, tag="c_xT")
                nc.sync.dma_start(out=cx,
                                  in_=d_xT[sl].rearrange("t d b -> d t b"))
                cxp = cp.tile([D, CT, B], F16, tag="c_xpT")
                nc.sync.dma_start(out=cxp,
                                  in_=d_xpT[sl].rearrange("t d b -> d t b"))
                cnz = cp.tile([D, CT, B], F32, tag="c_nzT")
                nc.sync.dma_start(out=cnz,
                                  in_=d_nzT[sl].rearrange("t d b -> d t b"))
                cnx = cp.tile([B, CT, D], F32, tag="c_nx")
                nc.sync.dma_start(out=cnx,
                                  in_=d_nx[sl].rearrange("t b d -> b t d"))
                cout = cp.tile([B, CT, D], F32, tag="c_out")
                chunk_tiles[c] = (cx, cxp, cnz, cnx, cout)

            def store_chunk(c):
                sl = slice(c * CT, (c + 1) * CT)
                nc.sync.dma_start(out=d_out[sl].rearrange("t b d -> b t d"),
                                  in_=chunk_tiles[c][4])

            def mm(out_ap, lhsT, rhs, start, stop):
                nc.tensor.matmul(out_ap, lhsT, rhs, start=start, stop=stop)

            def emit_hi(s):
                c, t = divmod(s, CT)
                cx, cxp = chunk_tiles[c][0], chunk_tiles[c][1]
                pc, pp = s % 2, 1 - s % 2
                # stage fresh xT(t) (with ones row at D) and pxT(t)
                nc.vector.tensor_copy(xh_rhs[0:D, pc, :], cx[:, t, :])
                nc.vector.tensor_copy(px_rhs[:, pc, :], cxp[:, t, :])
                ps = pp2.tile([128, 2 * B], F32, tag="hi_ps")
                for m in range(2):
                    o = ps[:, m * B:(m + 1) * B]
                    msl = slice(m * 128, (m + 1) * 128)
                    mm(o, w_hh_xh[:, 0, msl], hi_sb[:, pp, 0:B], True, False)
                    mm(o, w_hh_xh[:, 1, msl], hi_sb[:, pp, B:2 * B], False, False)
                    mm(o, w_xh[:, msl], xh_rhs[:, pc, :], False, True)
                nc.scalar.activation(hi_sb[:, pc, :], ps,
                                     mybir.ActivationFunctionType.Tanh)

            def emit_hl(s):
                # u(t-1) arrives last (off the exp/mul chain), so its two
                # matmuls go at the very end: only 2 matmuls then sit between
                # u's arrival and the tanh. Single accumulation group (one
                # start / one stop) since start=True re-zeroes the whole
                # 2KB zero region.
                pc, pp = s % 2, 1 - s % 2
                ps = pp2.tile([128, 2 * B], F32, tag="hl_ps")
                for m in range(2):
                    o = ps[:, m * B:(m + 1) * B]
                    msl = slice(m * 128, (m + 1) * 128)
                    mm(o, w_hh_hz[:, 0, msl], hl_sb[:, pp, 0:B], m == 0, False)
                    mm(o, w_hh_hz[:, 1, msl], hl_sb[:, pp, B:2 * B], False, False)
                    mm(o, w_hz1[:, 0, msl], hi_sb[:, pc, 0:B], False, False)
                    mm(o, w_hz1[:, 1, msl], hi_sb[:, pc, B:2 * B], False, False)
                for m in range(2):
                    o = ps[:, m * B:(m + 1) * B]
                    msl = slice(m * 128, (m + 1) * 128)
                    mm(o, w_hz2[:, msl], u_rhs[:, pp, :], False, m == 1)
                nc.scalar.activation(hl_sb[:, pc, :], ps,
                                     mybir.ActivationFunctionType.Tanh)

            def emit_z(s):
                c, t = divmod(s, CT)
                cnz = chunk_tiles[c][2]
                pc = s % 2
                ps = pp1.tile([128, B], F32, tag="zt_ps")
                mm(ps, w_zmzl[:, 0, :], hl_sb[:, pc, 0:B], True, False)
                mm(ps, w_zmzl[:, 1, :], hl_sb[:, pc, B:2 * B], False, True)
                # u = exp(lv + b_zl) * nz feeds the (rewritten) hl
                # recurrence; full z = u + (mz + b_zm) only feeds ho.
                ez = ep.tile([D, B], F32, tag="ez")
                nc.scalar.activation(ez, ps[D:2 * D, :],
                                     mybir.ActivationFunctionType.Exp,
                                     bias=b_z[D:2 * D, :])
                nc.vector.tensor_mul(u_rhs[0:D, pc, :], ez, cnz[:, t, :])
                nc.vector.scalar_tensor_tensor(
                    zo_rhs[0:D, pc, :], ps[0:D, :], b_z[0:D, :],
                    u_rhs[0:D, pc, :],
                    mybir.AluOpType.add, mybir.AluOpType.add)

            def emit_ho(s):
                pc, pp = s % 2, 1 - s % 2
                ps = pp2.tile([128, 2 * B], F32, tag="ho_ps")
                for m in range(2):
                    o = ps[:, m * B:(m + 1) * B]
                    msl = slice(m * 128, (m + 1) * 128)
                    mm(o, w_hh_zh[:, 0, msl], ho_sb[:, pp, 0:B], True, False)
                    mm(o, w_hh_zh[:, 1, msl], ho_sb[:, pp, B:2 * B], False, False)
                    mm(o, w_zx2[:, msl], px_rhs[:, pc, :], False, False)
                    mm(o, w_zx1[:, msl], zo_rhs[:, pc, :], False, True)
                nc.scalar.activation(ho_sb[:, pc, :], ps,
                                     mybir.ActivationFunctionType.Tanh)

            def emit_xpred(s):
                c, t = divmod(s, CT)
                cnx, cout = chunk_tiles[c][3], chunk_tiles[c][4]
                pc = s % 2
                ps = pp1.tile([B, D], F32, tag="mx_ps")
                mm(ps, ho_sb[:, pc, 0:B], w_xm[:, 0, :], True, False)
                mm(ps, ho_sb[:, pc, B:2 * B], w_xm[:, 1, :], False, True)
                # x_pred = exp(mx)*nx' + (mx + b_xm); nx' pre-scaled by
                # exp(b_xm) on the host.
                ex = ep.tile([B, D], F32, tag="ex")
                nc.scalar.activation(ex, ps,
                                     mybir.ActivationFunctionType.Exp)
                # m2 reads PSUM (GPSIMD can't); products go to GPSIMD to
                # keep the DVE queue free for next-step staging copies.
                m2 = ep.tile([B, D], F32, tag="m2")
                nc.vector.tensor_add(m2, ps, b_xm)
                p1 = ep.tile([B, D], F32, tag="p1")
                nc.gpsimd.tensor_mul(p1, ex, cnx[:, t, :])
                nc.gpsimd.tensor_add(cout[:, t, :], p1, m2)

            from contextlib import ExitStack
            with ExitStack() as stk:
                if reps > 1:
                    stk.enter_context(tc.For_i(0, reps, 1))
                chunk_tiles.clear()
                load_chunk(0)
                for s in range(T):
                    c, t = divmod(s, CT)
                    if t == CT // 2 and c + 1 < nchunks:
                        load_chunk(c + 1)
                    emit_hi(s)
                    emit_hl(s)
                    emit_z(s)
                    if s > 0:
                        emit_ho(s - 1)
                        emit_xpred(s - 1)
                        if s % CT == 0:
                            store_chunk(c - 1)
                emit_ho(T - 1)
                emit_xpred(T - 1)
                store_chunk(nchunks - 1)

    nc.compile()
    return nc


def prep_inputs(x, h_in0, h_lat0, h_out0, z0, tmp0, noise_z, noise_x,
                W_xh_ih, b_xh_ih, W_xh_hh, b_xh_hh,
                W_hz_ih, b_hz_ih, W_hz_hh, b_hz_hh,
                W_zh_ih, b_zh_ih, W_zh_hh, b_zh_hh,
                W_zm, b_zm, W_zl, b_zl, W_xm, b_xm, T=N):
    """Host-side preprocessing; returns the per-core in_map list."""
    f16, f32 = np.float16, np.float32
    xprev = np.concatenate([tmp0[None], x[:-1]], axis=0)
    xT = np.ascontiguousarray(x.transpose(0, 2, 1)).astype(f16)      # [t,d,b]
    xpT = np.ascontiguousarray(xprev.transpose(0, 2, 1)).astype(f16)
    nzT = np.ascontiguousarray(noise_z.transpose(0, 2, 1)).astype(f32)
    nxs = np.ascontiguousarray(noise_x * np.exp(b_xm)[None, None, :]).astype(f32)

    # Rewritten hl recurrence: with z = u + mz + b_zm (u = exp(lv)*nz,
    # mz = hl @ W_zm.T), fold the mz feedback into the hl-hl weight so only
    # u sits on the critical path:
    #   hl' = tanh(hi' @ Wz1.T + u @ Wz2.T
    #              + hl @ (W_hz_hh + Wz2 @ W_zm).T + b_hz + Wz2 @ b_zm)
    Wz2 = W_hz_ih[:, H:]
    b_hz_eff = b_hz_ih + b_hz_hh + Wz2 @ b_zm
    shared = {
        "Wxh": np.concatenate([W_xh_ih.T, (b_xh_ih + b_xh_hh)[None, :]],
                              axis=0).astype(f16),
        "Whh_xh": np.ascontiguousarray(W_xh_hh.T).astype(f16),
        "Whz1": np.ascontiguousarray(W_hz_ih[:, :H].T).astype(f16),
        "Whz2": np.concatenate([Wz2.T, b_hz_eff[None, :]],
                               axis=0).astype(f16),
        "Whh_hz": np.ascontiguousarray((W_hz_hh + Wz2 @ W_zm).T).astype(f16),
        "Wzmzl": np.ascontiguousarray(
            np.concatenate([W_zm.T, W_zl.T], axis=1)).astype(f16),
        "Wzx1": np.concatenate(
            [W_zh_ih[:, :D].T, (b_zh_ih + b_zh_hh)[None, :]],
            axis=0).astype(f16),
        "Wzx2": np.ascontiguousarray(W_zh_ih[:, D:].T).astype(f16),
        "Whh_zh": np.ascontiguousarray(W_zh_hh.T).astype(f16),
        "Wxm": np.ascontiguousarray(W_xm.T).astype(f16),
        "bz": np.concatenate([b_zm, b_zl]).astype(f32).reshape(2 * D, 1),
        "bxm": np.broadcast_to(b_xm, (B, D)).astype(f32).copy(),
    }

    def pack_state(h):       # [b_full, H] -> per-core [128, 2*B] packed
        hT = h.T.astype(f16)                     # [H, b_full]
        return hT.reshape(2, 128, h.shape[0])    # [k, p, b]

    hi_p, hl_p, ho_p = pack_state(h_in0), pack_state(h_lat0), pack_state(h_out0)
    # u0 chosen so the rewritten recurrence reproduces the given z0 exactly:
    # u0 = z0 - mz(h_lat0) - b_zm
    u0 = z0 - h_lat0 @ W_zm.T - b_zm
    z0T = u0.T.astype(f16)                       # [D, b_full]

    in_maps = []
    for c in range(NCORES):
        bs = slice(c * B, (c + 1) * B)
        m = dict(shared)
        m["xT"] = np.ascontiguousarray(xT[:T, :, bs])
        m["xpT"] = np.ascontiguousarray(xpT[:T, :, bs])
        m["nzT"] = np.ascontiguousarray(nzT[:T, :, bs])
        m["nx"] = np.ascontiguousarray(nxs[:T, bs, :])
        m["hi0"] = np.ascontiguousarray(
            hi_p[:, :, bs].transpose(1, 0, 2).reshape(128, 2 * B))
        m["hl0"] = np.ascontiguousarray(
            hl_p[:, :, bs].transpose(1, 0, 2).reshape(128, 2 * B))
        m["ho0"] = np.ascontiguousarray(
            ho_p[:, :, bs].transpose(1, 0, 2).reshape(128, 2 * B))
        m["z0"] = np.ascontiguousarray(z0T[:, bs])
        in_maps.append(m)
    return in_maps


def _get_nc(T=N, CT=32):
    key = (T, CT)
    if key not in _CACHE:
        _CACHE[key] = build_bass(T, CT)
    return _CACHE[key]


def run_on_hw(in_maps, T=N, CT=32):
    nc = _get_nc(T, CT)
    res = bass_utils.run_bass_kernel_spmd(
        nc, in_maps, core_ids=list(range(NCORES)))
    return res.results


class Runner:
    """Persistent jitted SPMD executor for a built Bass module (jit traced
    once; subsequent calls only pay H2D + execute)."""

    def __init__(self, nc):
        import jax
        from jax.sharding import Mesh, PartitionSpec, NamedSharding
        from jax.experimental.shard_map import shard_map
        from concourse import bass2jax

        bass2jax.install_neuronx_cc_hook()
        self._jax = jax
        pname = nc.partition_id_tensor.name if nc.partition_id_tensor else None
        in_names, out_names, out_avals, zeros = [], [], [], []
        for alloc in nc.m.functions[0].allocations:
            if not isinstance(alloc, mybir.MemoryLocationSet):
                continue
            name = alloc.memorylocations[0].name
            if alloc.kind == "ExternalInput":
                if name != pname:
                    in_names.append(name)
            elif alloc.kind == "ExternalOutput":
                out_names.append(name)
                shape = tuple(alloc.tensor_shape)
                dtype = mybir.dt.np(alloc.dtype)
                out_avals.append(jax.core.ShapedArray(shape, dtype))
                zeros.append(np.zeros(shape, dtype))
        self.in_names = list(in_names)
        self.out_names = list(out_names)
        all_names = in_names + out_names
        if pname is not None:
            all_names = all_names + [pname]

        def _body(*args):
            operands = list(args)
            if pname is not None:
                operands.append(bass2jax.partition_id_tensor())
            outs = bass2jax._bass_exec_p.bind(
                *operands,
                out_avals=tuple(out_avals),
                in_names=tuple(all_names),
                out_names=tuple(out_names),
                lowering_input_output_aliases=(),
                sim_require_finite=True,
                sim_require_nnan=True,
                nc=nc,
            )
            return tuple(outs)

        self._body = _body
        devices = jax.devices()[:NCORES]
        self.mesh = Mesh(np.asarray(devices), ("core",))
        spec = PartitionSpec("core")
        self.sharding = NamedSharding(self.mesh, spec)
        nin = len(in_names) + len(zeros)
        self.fn = jax.jit(
            shard_map(_body, mesh=self.mesh, in_specs=(spec,) * nin,
                      out_specs=(spec,) * len(out_names), check_rep=False),
            keep_unused=True)
        self.dev_zeros = [
            jax.device_put(np.zeros((NCORES * z.shape[0], *z.shape[1:]),
                                    z.dtype), self.sharding)
            for z in zeros]
        self.out_shapes = [tuple(a.shape) for a in out_avals]

    def concat_inputs(self, in_maps):
        return [np.concatenate([np.asarray(m[n]) for m in in_maps], axis=0)
                for n in self.in_names]

    def stage(self, in_maps):
        return [self._jax.device_put(a, self.sharding)
                for a in self.concat_inputs(in_maps)]

    def __call__(self, staged):
        outs = self.fn(*staged, *self.dev_zeros)
        self._jax.block_until_ready(outs)
        return outs

    def make_loop_fn(self, iters):
        """Jitted fn chaining `iters` kernel executions inside one dispatch
        (for timing: slope over iters = per-exec device time)."""
        import jax
        from jax.experimental.shard_map import shard_map
        from jax.sharding import PartitionSpec

        nx_i = self.in_names.index("nx")
        out_i = self.out_names.index("out")
        nin = len(self.in_names)
        body_fn = self._body

        def _loop(*args):
            ins = list(args[:nin])
            zeros = list(args[nin:])

            def body(i, carry):
                a = list(ins)
                a[nx_i] = a[nx_i] + 0.0 * carry
                outs = body_fn(*a, *zeros)
                return outs[out_i]

            return (jax.lax.fori_loop(0, iters, body, zeros[out_i]),)

        spec = PartitionSpec("core")
        nargs = nin + len(self.dev_zeros)
        return jax.jit(
            shard_map(_loop, mesh=self.mesh, in_specs=(spec,) * nargs,
                      out_specs=(spec,), check_rep=False),
            keep_unused=True)

    def split(self, outs):
        res = []
        for c in range(NCORES):
            res.append({
                n: np.asarray(outs[i]).reshape(NCORES, *self.out_shapes[i])[c]
                for i, n in enumerate(self.out_names)})
        return res


def get_runner(T=N, CT=32):
    key = ("runner", T, CT)
    if key not in _CACHE:
        _CACHE[key] = Runner(_get_nc(T, CT))
    return _CACHE[key]


def kernel(**inputs):
    in_maps = prep_inputs(**inputs)
    r = get_runner()
    outs = r(r.concat_inputs(in_maps))
    results = r.split(outs)
    out = np.empty((N, N, D), dtype=np.float32)
    for c in range(NCORES):
        out[:, c * B:(c + 1) * B, :] = results[c]["out"]
    return out



# revision 4
# speedup vs baseline: 167.6920x; 1.0080x over previous
"""Trainium2 Bass kernel for the NeuralVolatilityModel recurrence.

Strategy
--------
Data-parallel over batch (dim 1 of x): 8 cores x 32 batch each. The time
loop (256 steps) is sequential per core. States are kept dim-major in SBUF
([hidden_dim, batch]) so every RNN GEMM is `out[o, b] = W[o, i] @ h[i, b]`
with lhsT = W^T (stationary weights, prepared host-side in fp16) and
rhs = state (32 columns). Outputs land in PSUM dim-major, so the state
layout is self-consistent across steps: no transposes in the loop.

Host-side prep removes all on-chip data shuffling:
  * x, prev_x (with tmp0 folded in), noise_z are pre-transposed to
    [t, dim, batch]; noise_x is pre-scaled by exp(b_xm).
  * weights are transposed/concatenated; RNN biases are folded into an
    extra all-ones contraction row where a K slot is free, the rest are
    applied via the activation bias operand.

Per step and core, PE runs 26 LDWEIGHTS+MATMUL pairs (fp16, fp32 accum),
ACT runs tanh/exp, DVE does the reparameterization epilogues.

The kernel is latency-bound on the serial z-chain
(tanh_hl -> z GEMM -> exp -> mul -> hl GEMM); the u-dependent matmuls of
the hl GEMM are ordered last so only two matmuls separate u's arrival
from the tanh.
"""

import numpy as np

import concourse.bass as bass
import concourse.tile as tile
from concourse import bacc, mybir
from concourse import bass_utils
from concourse.bass import ts

F16 = mybir.dt.float16
F32 = mybir.dt.float32

N = 256          # time steps == full batch
D = 64           # input dim == latent dim
H = 256          # hidden dim
NCORES = 8
B = N // NCORES  # batch per core = 32

_CACHE = {}


def build_bass(T=N, CT=32, reps=1):
    """Build the Bass module for T time steps, chunked CT steps per loop
    iteration. reps>1 re-runs the whole computation (for device timing by
    slope). Returns the compiled Bacc object."""
    assert T % CT == 0 and CT % 2 == 0
    nchunks = T // CT

    nc = bacc.Bacc("TRN2", target_bir_lowering=False, debug=False,
                   enable_asserts=False, num_devices=NCORES)

    # ---- DRAM I/O (per-core shapes) ----
    d_xT = nc.dram_tensor("xT", [T, D, B], F16, kind="ExternalInput").ap()
    d_xpT = nc.dram_tensor("xpT", [T, D, B], F16, kind="ExternalInput").ap()
    d_nzT = nc.dram_tensor("nzT", [T, D, B], F16, kind="ExternalInput").ap()
    d_nx = nc.dram_tensor("nx", [T, B, D], F32, kind="ExternalInput").ap()

    d_Wxh = nc.dram_tensor("Wxh", [D + 1, H], F16, kind="ExternalInput").ap()
    d_Whh_xh = nc.dram_tensor("Whh_xh", [H, H], F16, kind="ExternalInput").ap()
    d_Whz1 = nc.dram_tensor("Whz1", [H, H], F16, kind="ExternalInput").ap()
    d_Whz2 = nc.dram_tensor("Whz2", [D + 1, H], F16, kind="ExternalInput").ap()
    d_Whh_hz = nc.dram_tensor("Whh_hz", [H, H], F16, kind="ExternalInput").ap()
    d_Wzmzl = nc.dram_tensor("Wzmzl", [H, 2 * D], F16, kind="ExternalInput").ap()
    d_Wzx1 = nc.dram_tensor("Wzx1", [D + 1, H], F16, kind="ExternalInput").ap()
    d_Wzx2 = nc.dram_tensor("Wzx2", [D, H], F16, kind="ExternalInput").ap()
    d_Whh_zh = nc.dram_tensor("Whh_zh", [H, H], F16, kind="ExternalInput").ap()
    d_Wxm = nc.dram_tensor("Wxm", [H, D], F16, kind="ExternalInput").ap()

    d_bz = nc.dram_tensor("bz", [2 * D, 1], F32, kind="ExternalInput").ap()
    d_bxm = nc.dram_tensor("bxm", [B, D], F32, kind="ExternalInput").ap()

    d_hi0 = nc.dram_tensor("hi0", [128, 2 * B], F16, kind="ExternalInput").ap()
    d_hl0 = nc.dram_tensor("hl0", [128, 2 * B], F16, kind="ExternalInput").ap()
    d_ho0 = nc.dram_tensor("ho0", [128, 2 * B], F16, kind="ExternalInput").ap()
    d_z0 = nc.dram_tensor("z0", [D, B], F16, kind="ExternalInput").ap()

    d_out = nc.dram_tensor("out", [T, B, D], F32, kind="ExternalOutput").ap()

    with tile.TileContext(nc) as tc:
        with (
            tc.tile_pool(name="weights", bufs=1) as wp,
            tc.tile_pool(name="states", bufs=1) as sp,
            tc.tile_pool(name="chunks", bufs=2) as cp,
            tc.tile_pool(name="epil", bufs=3) as ep,
            tc.tile_pool(name="ps2", bufs=2, space="PSUM") as pp2,
            tc.tile_pool(name="ps1", bufs=1, space="PSUM") as pp1,
        ):
            # ---- persistent weights ----
            w_xh = wp.tile([D + 1, H], F16, tag="w_xh")
            nc.sync.dma_start(out=w_xh, in_=d_Wxh)
            # [H, H] weights stored as [128, 2, H]: k-tile index on free dim
            w_hh_xh = wp.tile([128, 2, H], F16, tag="w_hh_xh")
            nc.sync.dma_start(
                out=w_hh_xh, in_=d_Whh_xh.rearrange("(k p) m -> p k m", p=128))
            w_hz1 = wp.tile([128, 2, H], F16, tag="w_hz1")
            nc.sync.dma_start(
                out=w_hz1, in_=d_Whz1.rearrange("(k p) m -> p k m", p=128))
            w_hz2 = wp.tile([D + 1, H], F16, tag="w_hz2")
            nc.sync.dma_start(out=w_hz2, in_=d_Whz2)
            w_hh_hz = wp.tile([128, 2, H], F16, tag="w_hh_hz")
            nc.sync.dma_start(
                out=w_hh_hz, in_=d_Whh_hz.rearrange("(k p) m -> p k m", p=128))
            w_zmzl = wp.tile([128, 2, 2 * D], F16, tag="w_zmzl")
            nc.sync.dma_start(
                out=w_zmzl, in_=d_Wzmzl.rearrange("(k p) m -> p k m", p=128))
            w_zx1 = wp.tile([D + 1, H], F16, tag="w_zx1")
            nc.sync.dma_start(out=w_zx1, in_=d_Wzx1)
            w_zx2 = wp.tile([D, H], F16, tag="w_zx2")
            nc.sync.dma_start(out=w_zx2, in_=d_Wzx2)
            w_hh_zh = wp.tile([128, 2, H], F16, tag="w_hh_zh")
            nc.sync.dma_start(
                out=w_hh_zh, in_=d_Whh_zh.rearrange("(k p) m -> p k m", p=128))
            w_xm = wp.tile([128, 2, D], F16, tag="w_xm")
            nc.sync.dma_start(
                out=w_xm, in_=d_Wxm.rearrange("(k p) m -> p k m", p=128))

            b_z = wp.tile([2 * D, 1], F32, tag="b_z")
            nc.sync.dma_start(out=b_z, in_=d_bz)
            b_xm = wp.tile([B, D], F32, tag="b_xm")
            nc.sync.dma_start(out=b_xm, in_=d_bxm)

            # ---- persistent states, parity 0/1 (step t writes t%2) ----
            hi_sb = sp.tile([128, 2, 2 * B], F16, tag="hi_sb")
            hl_sb = sp.tile([128, 2, 2 * B], F16, tag="hl_sb")
            ho_sb = sp.tile([128, 2, 2 * B], F16, tag="ho_sb")
            u_rhs = sp.tile([D + 1, 2, B], F16, tag="u_rhs")     # u | ones
            zo_rhs = sp.tile([D + 1, 2, B], F16, tag="zo_rhs")   # z | ones
            px_rhs = sp.tile([D, 2, B], F16, tag="px_rhs")       # pxT
            xh_rhs = sp.tile([D + 1, 2, B], F16, tag="xh_rhs")   # xT | ones

            nc.vector.memset(u_rhs[D:D + 1, :, :], 1.0)
            nc.vector.memset(zo_rhs[D:D + 1, :, :], 1.0)
            nc.vector.memset(xh_rhs[D:D + 1, :, :], 1.0)

            # initial states -> parity 1 (step 0 reads parity 1)
            nc.sync.dma_start(out=hi_sb[:, 1, :], in_=d_hi0)
            nc.sync.dma_start(out=hl_sb[:, 1, :], in_=d_hl0)
            nc.sync.dma_start(out=ho_sb[:, 1, :], in_=d_ho0)
            nc.sync.dma_start(out=u_rhs[0:D, 1, :], in_=d_z0)

            chunk_tiles = {}

            def load_chunk(c):
                sl = slice(c * CT, (c + 1) * CT)
                cx = cp.tile([D, CT, B], F16, tag="c_xT")
                nc.sync.dma_start(out=cx,
                                  in_=d_xT[sl].rearrange("t d b -> d t b"))
                cxp = cp.tile([D, CT, B], F16, tag="c_xpT")
                nc.sync.dma_start(out=cxp,
                                  in_=d_xpT[sl].rearrange("t d b -> d t b"))
                cnz = cp.tile([D, CT, B], F16, tag="c_nzT")
                nc.sync.dma_start(out=cnz,
                                  in_=d_nzT[sl].rearrange("t d b -> d t b"))
                cnx = cp.tile([B, CT, D], F32, tag="c_nx")
                nc.sync.dma_start(out=cnx,
                                  in_=d_nx[sl].rearrange("t b d -> b t d"))
                cout = cp.tile([B, CT, D], F32, tag="c_out")
                chunk_tiles[c] = (cx, cxp, cnz, cnx, cout)

            def store_chunk(c):
                sl = slice(c * CT, (c + 1) * CT)
                nc.sync.dma_start(out=d_out[sl].rearrange("t b d -> b t d"),
                                  in_=chunk_tiles[c][4])

            def mm(out_ap, lhsT, rhs, start, stop):
                nc.tensor.matmul(out_ap, lhsT, rhs, start=start, stop=stop)

            def emit_hi(s):
                c, t = divmod(s, CT)
                cx, cxp = chunk_tiles[c][0], chunk_tiles[c][1]
                pc, pp = s % 2, 1 - s % 2
                # stage fresh xT(t) (with ones row at D) and pxT(t)
                nc.vector.tensor_copy(xh_rhs[0:D, pc, :], cx[:, t, :])
                nc.vector.tensor_copy(px_rhs[:, pc, :], cxp[:, t, :])
                ps = pp2.tile([128, 2 * B], F32, tag="hi_ps")
                for m in range(2):
                    o = ps[:, m * B:(m + 1) * B]
                    msl = slice(m * 128, (m + 1) * 128)
                    mm(o, w_hh_xh[:, 0, msl], hi_sb[:, pp, 0:B], True, False)
                    mm(o, w_hh_xh[:, 1, msl], hi_sb[:, pp, B:2 * B], False, False)
                    mm(o, w_xh[:, msl], xh_rhs[:, pc, :], False, True)
                nc.scalar.activation(hi_sb[:, pc, :], ps,
                                     mybir.ActivationFunctionType.Tanh)

            def emit_hl(s):
                # u(t-1) arrives last (off the exp/mul chain), so its two
                # matmuls go at the very end: only 2 matmuls then sit between
                # u's arrival and the tanh. Single accumulation group (one
                # start / one stop) since start=True re-zeroes the whole
                # 2KB zero region.
                pc, pp = s % 2, 1 - s % 2
                ps = pp2.tile([128, 2 * B], F32, tag="hl_ps")
                for m in range(2):
                    o = ps[:, m * B:(m + 1) * B]
                    msl = slice(m * 128, (m + 1) * 128)
                    mm(o, w_hh_hz[:, 0, msl], hl_sb[:, pp, 0:B], m == 0, False)
                    mm(o, w_hh_hz[:, 1, msl], hl_sb[:, pp, B:2 * B], False, False)
                    mm(o, w_hz1[:, 0, msl], hi_sb[:, pc, 0:B], False, False)
                    mm(o, w_hz1[:, 1, msl], hi_sb[:, pc, B:2 * B], False, False)
                for m in range(2):
                    o = ps[:, m * B:(m + 1) * B]
                    msl = slice(m * 128, (m + 1) * 128)
                    mm(o, w_hz2[:, msl], u_rhs[:, pp, :], False, m == 1)
                nc.scalar.activation(hl_sb[:, pc, :], ps,
                                     mybir.ActivationFunctionType.Tanh)

            def emit_z(s):
                c, t = divmod(s, CT)
                cnz = chunk_tiles[c][2]
                pc = s % 2
                # lv (chain-critical: feeds exp -> u -> hl(t+1)) gets its own
                # PSUM bank and M=64 matmuls so exp waits only on these two;
                # mz (only feeds zo -> ho, one step of slack) goes separately.
                lv_ps = pp1.tile([D, B], F32, tag="lv_ps")
                mm(lv_ps, w_zmzl[:, 0, D:2 * D], hl_sb[:, pc, 0:B], True, False)
                mm(lv_ps, w_zmzl[:, 1, D:2 * D], hl_sb[:, pc, B:2 * B],
                   False, True)
                ez = ep.tile([D, B], F16, tag="ez")
                nc.scalar.activation(ez, lv_ps,
                                     mybir.ActivationFunctionType.Exp,
                                     bias=b_z[D:2 * D, :])
                nc.vector.tensor_mul(u_rhs[0:D, pc, :], ez, cnz[:, t, :])
                mz_ps = pp1.tile([D, B], F32, tag="mz_ps")
                mm(mz_ps, w_zmzl[:, 0, 0:D], hl_sb[:, pc, 0:B], True, False)
                mm(mz_ps, w_zmzl[:, 1, 0:D], hl_sb[:, pc, B:2 * B],
                   False, True)
                nc.vector.scalar_tensor_tensor(
                    zo_rhs[0:D, pc, :], mz_ps, b_z[0:D, :],
                    u_rhs[0:D, pc, :],
                    mybir.AluOpType.add, mybir.AluOpType.add)

            def emit_ho(s):
                pc, pp = s % 2, 1 - s % 2
                ps = pp1.tile([128, 2 * B], F32, tag="ho_ps")
                for m in range(2):
                    o = ps[:, m * B:(m + 1) * B]
                    msl = slice(m * 128, (m + 1) * 128)
                    mm(o, w_hh_zh[:, 0, msl], ho_sb[:, pp, 0:B], True, False)
                    mm(o, w_hh_zh[:, 1, msl], ho_sb[:, pp, B:2 * B], False, False)
                    mm(o, w_zx2[:, msl], px_rhs[:, pc, :], False, False)
                    mm(o, w_zx1[:, msl], zo_rhs[:, pc, :], False, True)
                nc.scalar.activation(ho_sb[:, pc, :], ps,
                                     mybir.ActivationFunctionType.Tanh)

            def emit_xpred(s):
                c, t = divmod(s, CT)
                cnx, cout = chunk_tiles[c][3], chunk_tiles[c][4]
                pc = s % 2
                ps = pp1.tile([B, D], F32, tag="mx_ps")
                mm(ps, ho_sb[:, pc, 0:B], w_xm[:, 0, :], True, False)
                mm(ps, ho_sb[:, pc, B:2 * B], w_xm[:, 1, :], False, True)
                # x_pred = exp(mx)*nx' + (mx + b_xm); nx' pre-scaled by
                # exp(b_xm) on the host.
                ex = ep.tile([B, D], F32, tag="ex")
                nc.scalar.activation(ex, ps,
                                     mybir.ActivationFunctionType.Exp)
                # m2 reads PSUM (GPSIMD can't); products go to GPSIMD to
                # keep the DVE queue free for next-step staging copies.
                m2 = ep.tile([B, D], F32, tag="m2")
                nc.vector.tensor_add(m2, ps, b_xm)
                p1 = ep.tile([B, D], F32, tag="p1")
                nc.gpsimd.tensor_mul(p1, ex, cnx[:, t, :])
                nc.gpsimd.tensor_add(cout[:, t, :], p1, m2)

            from contextlib import ExitStack
            with ExitStack() as stk:
                if reps > 1:
                    stk.enter_context(tc.For_i(0, reps, 1))
                chunk_tiles.clear()
                load_chunk(0)
                for s in range(T):
                    c, t = divmod(s, CT)
                    if t == CT // 2 and c + 1 < nchunks:
                        load_chunk(c + 1)
                    emit_hi(s)
                    emit_hl(s)
                    emit_z(s)
                    if s > 0:
                        emit_ho(s - 1)
                        emit_xpred(s - 1)
                        if s % CT == 0:
                            store_chunk(c - 1)
                emit_ho(T - 1)
                emit_xpred(T - 1)
                store_chunk(nchunks - 1)

    nc.compile()
    return nc


def prep_inputs(x, h_in0, h_lat0, h_out0, z0, tmp0, noise_z, noise_x,
                W_xh_ih, b_xh_ih, W_xh_hh, b_xh_hh,
                W_hz_ih, b_hz_ih, W_hz_hh, b_hz_hh,
                W_zh_ih, b_zh_ih, W_zh_hh, b_zh_hh,
                W_zm, b_zm, W_zl, b_zl, W_xm, b_xm, T=N):
    """Host-side preprocessing; returns the per-core in_map list."""
    f16, f32 = np.float16, np.float32
    xprev = np.concatenate([tmp0[None], x[:-1]], axis=0)
    xT = np.ascontiguousarray(x.transpose(0, 2, 1)).astype(f16)      # [t,d,b]
    xpT = np.ascontiguousarray(xprev.transpose(0, 2, 1)).astype(f16)
    nzT = np.ascontiguousarray(noise_z.transpose(0, 2, 1)).astype(f16)
    nxs = np.ascontiguousarray(noise_x * np.exp(b_xm)[None, None, :]).astype(f32)

    # Rewritten hl recurrence: with z = u + mz + b_zm (u = exp(lv)*nz,
    # mz = hl @ W_zm.T), fold the mz feedback into the hl-hl weight so only
    # u sits on the critical path:
    #   hl' = tanh(hi' @ Wz1.T + u @ Wz2.T
    #              + hl @ (W_hz_hh + Wz2 @ W_zm).T + b_hz + Wz2 @ b_zm)
    Wz2 = W_hz_ih[:, H:]
    b_hz_eff = b_hz_ih + b_hz_hh + Wz2 @ b_zm
    shared = {
        "Wxh": np.concatenate([W_xh_ih.T, (b_xh_ih + b_xh_hh)[None, :]],
                              axis=0).astype(f16),
        "Whh_xh": np.ascontiguousarray(W_xh_hh.T).astype(f16),
        "Whz1": np.ascontiguousarray(W_hz_ih[:, :H].T).astype(f16),
        "Whz2": np.concatenate([Wz2.T, b_hz_eff[None, :]],
                               axis=0).astype(f16),
        "Whh_hz": np.ascontiguousarray((W_hz_hh + Wz2 @ W_zm).T).astype(f16),
        "Wzmzl": np.ascontiguousarray(
            np.concatenate([W_zm.T, W_zl.T], axis=1)).astype(f16),
        "Wzx1": np.concatenate(
            [W_zh_ih[:, :D].T, (b_zh_ih + b_zh_hh)[None, :]],
            axis=0).astype(f16),
        "Wzx2": np.ascontiguousarray(W_zh_ih[:, D:].T).astype(f16),
        "Whh_zh": np.ascontiguousarray(W_zh_hh.T).astype(f16),
        "Wxm": np.ascontiguousarray(W_xm.T).astype(f16),
        "bz": np.concatenate([b_zm, b_zl]).astype(f32).reshape(2 * D, 1),
        "bxm": np.broadcast_to(b_xm, (B, D)).astype(f32).copy(),
    }

    def pack_state(h):       # [b_full, H] -> per-core [128, 2*B] packed
        hT = h.T.astype(f16)                     # [H, b_full]
        return hT.reshape(2, 128, h.shape[0])    # [k, p, b]

    hi_p, hl_p, ho_p = pack_state(h_in0), pack_state(h_lat0), pack_state(h_out0)
    # u0 chosen so the rewritten recurrence reproduces the given z0 exactly:
    # u0 = z0 - mz(h_lat0) - b_zm
    u0 = z0 - h_lat0 @ W_zm.T - b_zm
    z0T = u0.T.astype(f16)                       # [D, b_full]

    in_maps = []
    for c in range(NCORES):
        bs = slice(c * B, (c + 1) * B)
        m = dict(shared)
        m["xT"] = np.ascontiguousarray(xT[:T, :, bs])
        m["xpT"] = np.ascontiguousarray(xpT[:T, :, bs])
        m["nzT"] = np.ascontiguousarray(nzT[:T, :, bs])
        m["nx"] = np.ascontiguousarray(nxs[:T, bs, :])
        m["hi0"] = np.ascontiguousarray(
            hi_p[:, :, bs].transpose(1, 0, 2).reshape(128, 2 * B))
        m["hl0"] = np.ascontiguousarray(
            hl_p[:, :, bs].transpose(1, 0, 2).reshape(128, 2 * B))
        m["ho0"] = np.ascontiguousarray(
            ho_p[:, :, bs].transpose(1, 0, 2).reshape(128, 2 * B))
        m["z0"] = np.ascontiguousarray(z0T[:, bs])
        in_maps.append(m)
    return in_maps


def _get_nc(T=N, CT=32):
    key = (T, CT)
    if key not in _CACHE:
        _CACHE[key] = build_bass(T, CT)
    return _CACHE[key]


def run_on_hw(in_maps, T=N, CT=32):
    nc = _get_nc(T, CT)
    res = bass_utils.run_bass_kernel_spmd(
        nc, in_maps, core_ids=list(range(NCORES)))
    return res.results


class Runner:
    """Persistent jitted SPMD executor for a built Bass module (jit traced
    once; subsequent calls only pay H2D + execute)."""

    def __init__(self, nc):
        import jax
        from jax.sharding import Mesh, PartitionSpec, NamedSharding
        from jax.experimental.shard_map import shard_map
        from concourse import bass2jax

        bass2jax.install_neuronx_cc_hook()
        self._jax = jax
        pname = nc.partition_id_tensor.name if nc.partition_id_tensor else None
        in_names, out_names, out_avals, zeros = [], [], [], []
        for alloc in nc.m.functions[0].allocations:
            if not isinstance(alloc, mybir.MemoryLocationSet):
                continue
            name = alloc.memorylocations[0].name
            if alloc.kind == "ExternalInput":
                if name != pname:
                    in_names.append(name)
            elif alloc.kind == "ExternalOutput":
                out_names.append(name)
                shape = tuple(alloc.tensor_shape)
                dtype = mybir.dt.np(alloc.dtype)
                out_avals.append(jax.core.ShapedArray(shape, dtype))
                zeros.append(np.zeros(shape, dtype))
        self.in_names = list(in_names)
        self.out_names = list(out_names)
        all_names = in_names + out_names
        if pname is not None:
            all_names = all_names + [pname]

        def _body(*args):
            operands = list(args)
            if pname is not None:
                operands.append(bass2jax.partition_id_tensor())
            outs = bass2jax._bass_exec_p.bind(
                *operands,
                out_avals=tuple(out_avals),
                in_names=tuple(all_names),
                out_names=tuple(out_names),
                lowering_input_output_aliases=(),
                sim_require_finite=True,
                sim_require_nnan=True,
                nc=nc,
            )
            return tuple(outs)

        self._body = _body
        devices = jax.devices()[:NCORES]
        self.mesh = Mesh(np.asarray(devices), ("core",))
        spec = PartitionSpec("core")
        self.sharding = NamedSharding(self.mesh, spec)
        nin = len(in_names) + len(zeros)
        self.fn = jax.jit(
            shard_map(_body, mesh=self.mesh, in_specs=(spec,) * nin,
                      out_specs=(spec,) * len(out_names), check_rep=False),
            keep_unused=True)
        self.dev_zeros = [
            jax.device_put(np.zeros((NCORES * z.shape[0], *z.shape[1:]),
                                    z.dtype), self.sharding)
            for z in zeros]
        self.out_shapes = [tuple(a.shape) for a in out_avals]

    def concat_inputs(self, in_maps):
        return [np.concatenate([np.asarray(m[n]) for m in in_maps], axis=0)
                for n in self.in_names]

    def stage(self, in_maps):
        return [self._jax.device_put(a, self.sharding)
                for a in self.concat_inputs(in_maps)]

    def __call__(self, staged):
        outs = self.fn(*staged, *self.dev_zeros)
        self._jax.block_until_ready(outs)
        return outs

    def make_loop_fn(self, iters):
        """Jitted fn chaining `iters` kernel executions inside one dispatch
        (for timing: slope over iters = per-exec device time)."""
        import jax
        from jax.experimental.shard_map import shard_map
        from jax.sharding import PartitionSpec

        nx_i = self.in_names.index("nx")
        out_i = self.out_names.index("out")
        nin = len(self.in_names)
        body_fn = self._body

        def _loop(*args):
            ins = list(args[:nin])
            zeros = list(args[nin:])

            def body(i, carry):
                a = list(ins)
                a[nx_i] = a[nx_i] + 0.0 * carry
                outs = body_fn(*a, *zeros)
                return outs[out_i]

            return (jax.lax.fori_loop(0, iters, body, zeros[out_i]),)

        spec = PartitionSpec("core")
        nargs = nin + len(self.dev_zeros)
        return jax.jit(
            shard_map(_loop, mesh=self.mesh, in_specs=(spec,) * nargs,
                      out_specs=(spec,), check_rep=False),
            keep_unused=True)

    def split(self, outs):
        res = []
        for c in range(NCORES):
            res.append({
                n: np.asarray(outs[i]).reshape(NCORES, *self.out_shapes[i])[c]
                for i, n in enumerate(self.out_names)})
        return res


def get_runner(T=N, CT=32):
    key = ("runner", T, CT)
    if key not in _CACHE:
        _CACHE[key] = Runner(_get_nc(T, CT))
    return _CACHE[key]


def kernel(**inputs):
    in_maps = prep_inputs(**inputs)
    r = get_runner()
    outs = r(r.concat_inputs(in_maps))
    results = r.split(outs)
    out = np.empty((N, N, D), dtype=np.float32)
    for c in range(NCORES):
        out[:, c * B:(c + 1) * B, :] = results[c]["out"]
    return out



# revision 5
# speedup vs baseline: 168.5127x; 1.0049x over previous
"""Trainium2 Bass kernel for the NeuralVolatilityModel recurrence.

Strategy
--------
Data-parallel over batch (dim 1 of x): 8 cores x 32 batch each. The time
loop (256 steps) is sequential per core. States are kept dim-major in SBUF
([hidden_dim, batch]) so every RNN GEMM is `out[o, b] = W[o, i] @ h[i, b]`
with lhsT = W^T (stationary weights, prepared host-side in fp16) and
rhs = state (32 columns). Outputs land in PSUM dim-major, so the state
layout is self-consistent across steps: no transposes in the loop.

Host-side prep removes all on-chip data shuffling:
  * x, prev_x (with tmp0 folded in), noise_z are pre-transposed to
    [t, dim, batch]; noise_x is pre-scaled by exp(b_xm).
  * weights are transposed/concatenated; RNN biases are folded into an
    extra all-ones contraction row where a K slot is free, the rest are
    applied via the activation bias operand.

Per step and core, PE runs 26 LDWEIGHTS+MATMUL pairs (fp16, fp32 accum),
ACT runs tanh/exp, DVE does the reparameterization epilogues.

The kernel is latency-bound on the serial z-chain
(tanh_hl -> z GEMM -> exp -> mul -> hl GEMM); the u-dependent matmuls of
the hl GEMM are ordered last so only two matmuls separate u's arrival
from the tanh.
"""

import numpy as np

import concourse.bass as bass
import concourse.tile as tile
from concourse import bacc, mybir
from concourse import bass_utils
from concourse.bass import ts

F16 = mybir.dt.float16
F32 = mybir.dt.float32

N = 256          # time steps == full batch
D = 64           # input dim == latent dim
H = 256          # hidden dim
NCORES = 8
B = N // NCORES  # batch per core = 32

_CACHE = {}


def build_bass(T=N, CT=32, reps=1):
    """Build the Bass module for T time steps, chunked CT steps per loop
    iteration. reps>1 re-runs the whole computation (for device timing by
    slope). Returns the compiled Bacc object."""
    assert T % CT == 0 and CT % 2 == 0
    nchunks = T // CT

    nc = bacc.Bacc("TRN2", target_bir_lowering=False, debug=False,
                   enable_asserts=False, num_devices=NCORES)

    # ---- DRAM I/O (per-core shapes) ----
    d_xT = nc.dram_tensor("xT", [T, D, B], F16, kind="ExternalInput").ap()
    d_xpT = nc.dram_tensor("xpT", [T, D, B], F16, kind="ExternalInput").ap()
    d_nzT = nc.dram_tensor("nzT", [T, D, B], F16, kind="ExternalInput").ap()
    d_nx = nc.dram_tensor("nx", [T, B, D], F32, kind="ExternalInput").ap()

    d_Wxh = nc.dram_tensor("Wxh", [D + 1, H], F16, kind="ExternalInput").ap()
    d_Whh_xh = nc.dram_tensor("Whh_xh", [H, H], F16, kind="ExternalInput").ap()
    d_Whz1 = nc.dram_tensor("Whz1", [H, H], F16, kind="ExternalInput").ap()
    d_Whz2 = nc.dram_tensor("Whz2", [D + 1, H], F16, kind="ExternalInput").ap()
    d_Whh_hz = nc.dram_tensor("Whh_hz", [H, H], F16, kind="ExternalInput").ap()
    d_Wzmzl = nc.dram_tensor("Wzmzl", [H, 2 * D], F16, kind="ExternalInput").ap()
    d_Wzx1 = nc.dram_tensor("Wzx1", [D + 1, H], F16, kind="ExternalInput").ap()
    d_Wzx2 = nc.dram_tensor("Wzx2", [D, H], F16, kind="ExternalInput").ap()
    d_Whh_zh = nc.dram_tensor("Whh_zh", [H, H], F16, kind="ExternalInput").ap()
    d_Wxm = nc.dram_tensor("Wxm", [H, D], F16, kind="ExternalInput").ap()

    d_bz = nc.dram_tensor("bz", [2 * D, 1], F32, kind="ExternalInput").ap()
    d_bxm = nc.dram_tensor("bxm", [B, 2 * D], F32, kind="ExternalInput").ap()

    d_hi0 = nc.dram_tensor("hi0", [128, 2 * B], F16, kind="ExternalInput").ap()
    d_hl0 = nc.dram_tensor("hl0", [128, 2 * B], F16, kind="ExternalInput").ap()
    d_ho0 = nc.dram_tensor("ho0", [128, 2 * B], F16, kind="ExternalInput").ap()
    d_z0 = nc.dram_tensor("z0", [D, B], F16, kind="ExternalInput").ap()

    d_out = nc.dram_tensor("out", [T, B, D], F32, kind="ExternalOutput").ap()

    with tile.TileContext(nc) as tc:
        with (
            tc.tile_pool(name="weights", bufs=1) as wp,
            tc.tile_pool(name="states", bufs=1) as sp,
            tc.tile_pool(name="chunks", bufs=2) as cp,
            tc.tile_pool(name="epil", bufs=3) as ep,
            tc.tile_pool(name="ps2", bufs=2, space="PSUM") as pp2,
            tc.tile_pool(name="ps1", bufs=1, space="PSUM") as pp1,
        ):
            # ---- persistent weights ----
            w_xh = wp.tile([D + 1, H], F16, tag="w_xh")
            nc.sync.dma_start(out=w_xh, in_=d_Wxh)
            # [H, H] weights stored as [128, 2, H]: k-tile index on free dim
            w_hh_xh = wp.tile([128, 2, H], F16, tag="w_hh_xh")
            nc.sync.dma_start(
                out=w_hh_xh, in_=d_Whh_xh.rearrange("(k p) m -> p k m", p=128))
            w_hz1 = wp.tile([128, 2, H], F16, tag="w_hz1")
            nc.sync.dma_start(
                out=w_hz1, in_=d_Whz1.rearrange("(k p) m -> p k m", p=128))
            w_hz2 = wp.tile([D + 1, H], F16, tag="w_hz2")
            nc.sync.dma_start(out=w_hz2, in_=d_Whz2)
            w_hh_hz = wp.tile([128, 2, H], F16, tag="w_hh_hz")
            nc.sync.dma_start(
                out=w_hh_hz, in_=d_Whh_hz.rearrange("(k p) m -> p k m", p=128))
            w_zmzl = wp.tile([128, 2, 2 * D], F16, tag="w_zmzl")
            nc.sync.dma_start(
                out=w_zmzl, in_=d_Wzmzl.rearrange("(k p) m -> p k m", p=128))
            w_zx1 = wp.tile([D + 1, H], F16, tag="w_zx1")
            nc.sync.dma_start(out=w_zx1, in_=d_Wzx1)
            w_zx2 = wp.tile([D, H], F16, tag="w_zx2")
            nc.sync.dma_start(out=w_zx2, in_=d_Wzx2)
            w_hh_zh = wp.tile([128, 2, H], F16, tag="w_hh_zh")
            nc.sync.dma_start(
                out=w_hh_zh, in_=d_Whh_zh.rearrange("(k p) m -> p k m", p=128))
            w_xm = wp.tile([128, 2, D], F16, tag="w_xm")
            nc.sync.dma_start(
                out=w_xm, in_=d_Wxm.rearrange("(k p) m -> p k m", p=128))

            b_z = wp.tile([2 * D, 1], F32, tag="b_z")
            nc.sync.dma_start(out=b_z, in_=d_bz)
            b_xm = wp.tile([B, 2 * D], F32, tag="b_xm")
            nc.sync.dma_start(out=b_xm, in_=d_bxm)

            # ---- persistent states, parity 0/1 (step t writes t%2) ----
            hi_sb = sp.tile([128, 2, 2 * B], F16, tag="hi_sb")
            hl_sb = sp.tile([128, 2, 2 * B], F16, tag="hl_sb")
            ho_sb = sp.tile([128, 2, 2 * B], F16, tag="ho_sb")
            u_rhs = sp.tile([D + 1, 2, B], F16, tag="u_rhs")     # u | ones
            zo_rhs = sp.tile([D + 1, 2, B], F16, tag="zo_rhs")   # z | ones
            px_rhs = sp.tile([D, 2, B], F16, tag="px_rhs")       # pxT
            xh_rhs = sp.tile([D + 1, 2, B], F16, tag="xh_rhs")   # xT | ones

            nc.vector.memset(u_rhs[D:D + 1, :, :], 1.0)
            nc.vector.memset(zo_rhs[D:D + 1, :, :], 1.0)
            nc.vector.memset(xh_rhs[D:D + 1, :, :], 1.0)

            # initial states -> parity 1 (step 0 reads parity 1)
            nc.sync.dma_start(out=hi_sb[:, 1, :], in_=d_hi0)
            nc.sync.dma_start(out=hl_sb[:, 1, :], in_=d_hl0)
            nc.sync.dma_start(out=ho_sb[:, 1, :], in_=d_ho0)
            nc.sync.dma_start(out=u_rhs[0:D, 1, :], in_=d_z0)

            chunk_tiles = {}

            def load_chunk(c):
                sl = slice(c * CT, (c + 1) * CT)
                cx = cp.tile([D, CT, B], F16, tag="c_xT")
                nc.sync.dma_start(out=cx,
                                  in_=d_xT[sl].rearrange("t d b -> d t b"))
                cxp = cp.tile([D, CT, B], F16, tag="c_xpT")
                nc.sync.dma_start(out=cxp,
                                  in_=d_xpT[sl].rearrange("t d b -> d t b"))
                cnz = cp.tile([D, CT, B], F16, tag="c_nzT")
                nc.sync.dma_start(out=cnz,
                                  in_=d_nzT[sl].rearrange("t d b -> d t b"))
                cnx = cp.tile([B, CT, D], F32, tag="c_nx")
                nc.sync.dma_start(out=cnx,
                                  in_=d_nx[sl].rearrange("t b d -> b t d"))
                cout = cp.tile([B, CT, D], F32, tag="c_out")
                chunk_tiles[c] = (cx, cxp, cnz, cnx, cout)

            def store_chunk(c):
                sl = slice(c * CT, (c + 1) * CT)
                nc.sync.dma_start(out=d_out[sl].rearrange("t b d -> b t d"),
                                  in_=chunk_tiles[c][4])

            def mm(out_ap, lhsT, rhs, start, stop):
                nc.tensor.matmul(out_ap, lhsT, rhs, start=start, stop=stop)

            def emit_hi(s):
                c, t = divmod(s, CT)
                cx, cxp = chunk_tiles[c][0], chunk_tiles[c][1]
                pc, pp = s % 2, 1 - s % 2
                # stage fresh xT(t) (with ones row at D) and pxT(t)
                nc.vector.tensor_copy(xh_rhs[0:D, pc, :], cx[:, t, :])
                nc.vector.tensor_copy(px_rhs[:, pc, :], cxp[:, t, :])
                ps = pp2.tile([128, 2 * B], F32, tag="hi_ps")
                for m in range(2):
                    o = ps[:, m * B:(m + 1) * B]
                    msl = slice(m * 128, (m + 1) * 128)
                    mm(o, w_hh_xh[:, 0, msl], hi_sb[:, pp, 0:B], True, False)
                    mm(o, w_hh_xh[:, 1, msl], hi_sb[:, pp, B:2 * B], False, False)
                    mm(o, w_xh[:, msl], xh_rhs[:, pc, :], False, True)
                nc.scalar.activation(hi_sb[:, pc, :], ps,
                                     mybir.ActivationFunctionType.Tanh)

            def emit_hl(s):
                # u(t-1) arrives last (off the exp/mul chain), so its two
                # matmuls go at the very end: only 2 matmuls then sit between
                # u's arrival and the tanh. Single accumulation group (one
                # start / one stop) since start=True re-zeroes the whole
                # 2KB zero region.
                pc, pp = s % 2, 1 - s % 2
                ps = pp2.tile([128, 2 * B], F32, tag="hl_ps")
                for m in range(2):
                    o = ps[:, m * B:(m + 1) * B]
                    msl = slice(m * 128, (m + 1) * 128)
                    mm(o, w_hh_hz[:, 0, msl], hl_sb[:, pp, 0:B], m == 0, False)
                    mm(o, w_hh_hz[:, 1, msl], hl_sb[:, pp, B:2 * B], False, False)
                    mm(o, w_hz1[:, 0, msl], hi_sb[:, pc, 0:B], False, False)
                    mm(o, w_hz1[:, 1, msl], hi_sb[:, pc, B:2 * B], False, False)
                for m in range(2):
                    o = ps[:, m * B:(m + 1) * B]
                    msl = slice(m * 128, (m + 1) * 128)
                    mm(o, w_hz2[:, msl], u_rhs[:, pp, :], False, m == 1)
                nc.scalar.activation(hl_sb[:, pc, :], ps,
                                     mybir.ActivationFunctionType.Tanh)

            def emit_z(s):
                c, t = divmod(s, CT)
                cnz = chunk_tiles[c][2]
                pc = s % 2
                # lv (chain-critical: feeds exp -> u -> hl(t+1)) gets its own
                # PSUM bank and M=64 matmuls so exp waits only on these two;
                # mz (only feeds zo -> ho, one step of slack) goes separately.
                lv_ps = pp1.tile([D, B], F32, tag="lv_ps")
                mm(lv_ps, w_zmzl[:, 0, D:2 * D], hl_sb[:, pc, 0:B], True, False)
                mm(lv_ps, w_zmzl[:, 1, D:2 * D], hl_sb[:, pc, B:2 * B],
                   False, True)
                ez = ep.tile([D, B], F16, tag="ez")
                nc.scalar.activation(ez, lv_ps,
                                     mybir.ActivationFunctionType.Exp,
                                     bias=b_z[D:2 * D, :])
                nc.vector.tensor_mul(u_rhs[0:D, pc, :], ez, cnz[:, t, :])
                mz_ps = pp1.tile([D, B], F32, tag="mz_ps")
                mm(mz_ps, w_zmzl[:, 0, 0:D], hl_sb[:, pc, 0:B], True, False)
                mm(mz_ps, w_zmzl[:, 1, 0:D], hl_sb[:, pc, B:2 * B],
                   False, True)
                nc.vector.scalar_tensor_tensor(
                    zo_rhs[0:D, pc, :], mz_ps, b_z[0:D, :],
                    u_rhs[0:D, pc, :],
                    mybir.AluOpType.add, mybir.AluOpType.add)

            def emit_ho(s):
                pc, pp = s % 2, 1 - s % 2
                ps = pp1.tile([128, 2 * B], F32, tag="ho_ps")
                for m in range(2):
                    o = ps[:, m * B:(m + 1) * B]
                    msl = slice(m * 128, (m + 1) * 128)
                    mm(o, w_hh_zh[:, 0, msl], ho_sb[:, pp, 0:B], True, False)
                    mm(o, w_hh_zh[:, 1, msl], ho_sb[:, pp, B:2 * B], False, False)
                    mm(o, w_zx2[:, msl], px_rhs[:, pc, :], False, False)
                    mm(o, w_zx1[:, msl], zo_rhs[:, pc, :], False, True)
                nc.scalar.activation(ho_sb[:, pc, :], ps,
                                     mybir.ActivationFunctionType.Tanh)

            mx_tiles = {}

            def emit_xpred(s):
                # pairs of steps share one [B, 2D] PSUM tile and flush
                # together: halves the ACT/DVE/GPSIMD epilogue op count at
                # only +43 ns filler granularity on the contended ACT queue
                # (the G=8 variant's 720 ns exp head-of-line blocked the
                # chain and lost).
                g, j = divmod(s, 2)
                pc = s % 2
                if j == 0:
                    mx_ps = pp1.tile([B, 2 * D], F32, tag="mx_ps")
                    mx_tiles[g] = mx_ps
                ps = mx_tiles[g]
                o = ps[:, j * D:(j + 1) * D]
                mm(o, ho_sb[:, pc, 0:B], w_xm[:, 0, :], j == 0, False)
                mm(o, ho_sb[:, pc, B:2 * B], w_xm[:, 1, :], False, j == 1)
                if j == 1:
                    flush_pair(g)

            def flush_pair(g):
                # x_pred = exp(mx)*nx' + (mx + b_xm) for 2 steps at once;
                # nx' pre-scaled by exp(b_xm) on the host.
                c, tt0 = divmod(2 * g, CT)
                cnx, cout = chunk_tiles[c][3], chunk_tiles[c][4]
                ps = mx_tiles.pop(g)
                ex = ep.tile([B, 2 * D], F32, tag="ex")
                nc.scalar.activation(ex, ps,
                                     mybir.ActivationFunctionType.Exp)
                m2 = ep.tile([B, 2 * D], F32, tag="m2")
                nc.vector.tensor_add(m2, ps, b_xm)
                p1 = ep.tile([B, 2 * D], F32, tag="p1")
                nc.gpsimd.tensor_mul(p1, ex, cnx[:, tt0:tt0 + 2, :])
                nc.gpsimd.tensor_add(cout[:, tt0:tt0 + 2, :], p1, m2)

            from contextlib import ExitStack
            with ExitStack() as stk:
                if reps > 1:
                    stk.enter_context(tc.For_i(0, reps, 1))
                chunk_tiles.clear()
                load_chunk(0)
                for s in range(T):
                    c, t = divmod(s, CT)
                    if t == CT // 2 and c + 1 < nchunks:
                        load_chunk(c + 1)
                    emit_hi(s)
                    emit_hl(s)
                    emit_z(s)
                    if s > 0:
                        emit_ho(s - 1)
                        emit_xpred(s - 1)
                        if s % CT == 0:
                            store_chunk(c - 1)
                emit_ho(T - 1)
                emit_xpred(T - 1)
                store_chunk(nchunks - 1)

    nc.compile()
    return nc


def prep_inputs(x, h_in0, h_lat0, h_out0, z0, tmp0, noise_z, noise_x,
                W_xh_ih, b_xh_ih, W_xh_hh, b_xh_hh,
                W_hz_ih, b_hz_ih, W_hz_hh, b_hz_hh,
                W_zh_ih, b_zh_ih, W_zh_hh, b_zh_hh,
                W_zm, b_zm, W_zl, b_zl, W_xm, b_xm, T=N):
    """Host-side preprocessing; returns the per-core in_map list."""
    f16, f32 = np.float16, np.float32
    xprev = np.concatenate([tmp0[None], x[:-1]], axis=0)
    xT = np.ascontiguousarray(x.transpose(0, 2, 1)).astype(f16)      # [t,d,b]
    xpT = np.ascontiguousarray(xprev.transpose(0, 2, 1)).astype(f16)
    nzT = np.ascontiguousarray(noise_z.transpose(0, 2, 1)).astype(f16)
    nxs = np.ascontiguousarray(noise_x * np.exp(b_xm)[None, None, :]).astype(f32)

    # Rewritten hl recurrence: with z = u + mz + b_zm (u = exp(lv)*nz,
    # mz = hl @ W_zm.T), fold the mz feedback into the hl-hl weight so only
    # u sits on the critical path:
    #   hl' = tanh(hi' @ Wz1.T + u @ Wz2.T
    #              + hl @ (W_hz_hh + Wz2 @ W_zm).T + b_hz + Wz2 @ b_zm)
    Wz2 = W_hz_ih[:, H:]
    b_hz_eff = b_hz_ih + b_hz_hh + Wz2 @ b_zm
    shared = {
        "Wxh": np.concatenate([W_xh_ih.T, (b_xh_ih + b_xh_hh)[None, :]],
                              axis=0).astype(f16),
        "Whh_xh": np.ascontiguousarray(W_xh_hh.T).astype(f16),
        "Whz1": np.ascontiguousarray(W_hz_ih[:, :H].T).astype(f16),
        "Whz2": np.concatenate([Wz2.T, b_hz_eff[None, :]],
                               axis=0).astype(f16),
        "Whh_hz": np.ascontiguousarray((W_hz_hh + Wz2 @ W_zm).T).astype(f16),
        "Wzmzl": np.ascontiguousarray(
            np.concatenate([W_zm.T, W_zl.T], axis=1)).astype(f16),
        "Wzx1": np.concatenate(
            [W_zh_ih[:, :D].T, (b_zh_ih + b_zh_hh)[None, :]],
            axis=0).astype(f16),
        "Wzx2": np.ascontiguousarray(W_zh_ih[:, D:].T).astype(f16),
        "Whh_zh": np.ascontiguousarray(W_zh_hh.T).astype(f16),
        "Wxm": np.ascontiguousarray(W_xm.T).astype(f16),
        "bz": np.concatenate([b_zm, b_zl]).astype(f32).reshape(2 * D, 1),
        "bxm": np.broadcast_to(np.tile(b_xm, 2), (B, 2 * D)).astype(f32).copy(),
    }

    def pack_state(h):       # [b_full, H] -> per-core [128, 2*B] packed
        hT = h.T.astype(f16)                     # [H, b_full]
        return hT.reshape(2, 128, h.shape[0])    # [k, p, b]

    hi_p, hl_p, ho_p = pack_state(h_in0), pack_state(h_lat0), pack_state(h_out0)
    # u0 chosen so the rewritten recurrence reproduces the given z0 exactly:
    # u0 = z0 - mz(h_lat0) - b_zm
    u0 = z0 - h_lat0 @ W_zm.T - b_zm
    z0T = u0.T.astype(f16)                       # [D, b_full]

    in_maps = []
    for c in range(NCORES):
        bs = slice(c * B, (c + 1) * B)
        m = dict(shared)
        m["xT"] = np.ascontiguousarray(xT[:T, :, bs])
        m["xpT"] = np.ascontiguousarray(xpT[:T, :, bs])
        m["nzT"] = np.ascontiguousarray(nzT[:T, :, bs])
        m["nx"] = np.ascontiguousarray(nxs[:T, bs, :])
        m["hi0"] = np.ascontiguousarray(
            hi_p[:, :, bs].transpose(1, 0, 2).reshape(128, 2 * B))
        m["hl0"] = np.ascontiguousarray(
            hl_p[:, :, bs].transpose(1, 0, 2).reshape(128, 2 * B))
        m["ho0"] = np.ascontiguousarray(
            ho_p[:, :, bs].transpose(1, 0, 2).reshape(128, 2 * B))
        m["z0"] = np.ascontiguousarray(z0T[:, bs])
        in_maps.append(m)
    return in_maps


def _get_nc(T=N, CT=32):
    key = (T, CT)
    if key not in _CACHE:
        _CACHE[key] = build_bass(T, CT)
    return _CACHE[key]


def run_on_hw(in_maps, T=N, CT=32):
    nc = _get_nc(T, CT)
    res = bass_utils.run_bass_kernel_spmd(
        nc, in_maps, core_ids=list(range(NCORES)))
    return res.results


class Runner:
    """Persistent jitted SPMD executor for a built Bass module (jit traced
    once; subsequent calls only pay H2D + execute)."""

    def __init__(self, nc):
        import jax
        from jax.sharding import Mesh, PartitionSpec, NamedSharding
        from jax.experimental.shard_map import shard_map
        from concourse import bass2jax

        bass2jax.install_neuronx_cc_hook()
        self._jax = jax
        pname = nc.partition_id_tensor.name if nc.partition_id_tensor else None
        in_names, out_names, out_avals, zeros = [], [], [], []
        for alloc in nc.m.functions[0].allocations:
            if not isinstance(alloc, mybir.MemoryLocationSet):
                continue
            name = alloc.memorylocations[0].name
            if alloc.kind == "ExternalInput":
                if name != pname:
                    in_names.append(name)
            elif alloc.kind == "ExternalOutput":
                out_names.append(name)
                shape = tuple(alloc.tensor_shape)
                dtype = mybir.dt.np(alloc.dtype)
                out_avals.append(jax.core.ShapedArray(shape, dtype))
                zeros.append(np.zeros(shape, dtype))
        self.in_names = list(in_names)
        self.out_names = list(out_names)
        all_names = in_names + out_names
        if pname is not None:
            all_names = all_names + [pname]

        def _body(*args):
            operands = list(args)
            if pname is not None:
                operands.append(bass2jax.partition_id_tensor())
            outs = bass2jax._bass_exec_p.bind(
                *operands,
                out_avals=tuple(out_avals),
                in_names=tuple(all_names),
                out_names=tuple(out_names),
                lowering_input_output_aliases=(),
                sim_require_finite=True,
                sim_require_nnan=True,
                nc=nc,
            )
            return tuple(outs)

        self._body = _body
        devices = jax.devices()[:NCORES]
        self.mesh = Mesh(np.asarray(devices), ("core",))
        spec = PartitionSpec("core")
        self.sharding = NamedSharding(self.mesh, spec)
        nin = len(in_names) + len(zeros)
        self.fn = jax.jit(
            shard_map(_body, mesh=self.mesh, in_specs=(spec,) * nin,
                      out_specs=(spec,) * len(out_names), check_rep=False),
            keep_unused=True)
        self.dev_zeros = [
            jax.device_put(np.zeros((NCORES * z.shape[0], *z.shape[1:]),
                                    z.dtype), self.sharding)
            for z in zeros]
        self.out_shapes = [tuple(a.shape) for a in out_avals]

    def concat_inputs(self, in_maps):
        return [np.concatenate([np.asarray(m[n]) for m in in_maps], axis=0)
                for n in self.in_names]

    def stage(self, in_maps):
        return [self._jax.device_put(a, self.sharding)
                for a in self.concat_inputs(in_maps)]

    def __call__(self, staged):
        outs = self.fn(*staged, *self.dev_zeros)
        self._jax.block_until_ready(outs)
        return outs

    def make_loop_fn(self, iters):
        """Jitted fn chaining `iters` kernel executions inside one dispatch
        (for timing: slope over iters = per-exec device time)."""
        import jax
        from jax.experimental.shard_map import shard_map
        from jax.sharding import PartitionSpec

        nx_i = self.in_names.index("nx")
        out_i = self.out_names.index("out")
        nin = len(self.in_names)
        body_fn = self._body

        def _loop(*args):
            ins = list(args[:nin])
            zeros = list(args[nin:])

            def body(i, carry):
                a = list(ins)
                a[nx_i] = a[nx_i] + 0.0 * carry
                outs = body_fn(*a, *zeros)
                return outs[out_i]

            return (jax.lax.fori_loop(0, iters, body, zeros[out_i]),)

        spec = PartitionSpec("core")
        nargs = nin + len(self.dev_zeros)
        return jax.jit(
            shard_map(_loop, mesh=self.mesh, in_specs=(spec,) * nargs,
                      out_specs=(spec,), check_rep=False),
            keep_unused=True)

    def split(self, outs):
        res = []
        for c in range(NCORES):
            res.append({
                n: np.asarray(outs[i]).reshape(NCORES, *self.out_shapes[i])[c]
                for i, n in enumerate(self.out_names)})
        return res


def get_runner(T=N, CT=32):
    key = ("runner", T, CT)
    if key not in _CACHE:
        _CACHE[key] = Runner(_get_nc(T, CT))
    return _CACHE[key]


def kernel(**inputs):
    in_maps = prep_inputs(**inputs)
    r = get_runner()
    outs = r(r.concat_inputs(in_maps))
    results = r.split(outs)
    out = np.empty((N, N, D), dtype=np.float32)
    for c in range(NCORES):
        out[:, c * B:(c + 1) * B, :] = results[c]["out"]
    return out

